# revision 7
# baseline (speedup 1.0000x reference)
"""BiMamba (fwd+bwd Mamba + merge) Trainium2 Bass kernel.

Sharding (8 cores): core = batch*4 + dir*2 + e_half.
Each core computes one (batch, direction) pair over 1024 of the 2048 d_inner
channels, in e-partition layout [e_p=128 x 8 tiles, t_free=1024].
bwd cores operate entirely in flipped time (host pre-flips x); the final
out_proj partial is un-flipped via a data-driven mask combine, then a 4-core
AllReduce produces the full (d, t) output on every core of the batch group.

Self-contained: hardcodes B=2, L=1024, D=1024, E=2048 (1024/core), N=16,
dt_rank=64, d_conv=4.
"""
import numpy as np

B, L, D = 2, 1024, 1024
E = 2048
EH = 1024            # channels per core (half of E)
N = 16
DTR = 64
K = 4                # d_conv
M_TILES = 8          # e-tiles per core
NB = 8               # n-plane batches
NPB = 2              # planes per batch
PL = L + 2           # plane stride with 2-col zero gap for the batched scan

_nc_cache = {}


def _build_nc():
    import concourse.bacc as bacc
    import concourse.mybir as mybir
    from concourse import tile

    f32, f16 = mybir.dt.float32, mybir.dt.float16
    Alu = mybir.AluOpType
    Act = mybir.ActivationFunctionType

    nc = bacc.Bacc("TRN2", target_bir_lowering=False, debug=False, num_devices=8)

    # ---- DRAM I/O ----
    xT_d = nc.dram_tensor("xT", [D, 3 + L], f16, kind="ExternalInput")
    wxiT_d = nc.dram_tensor("wxiT", [D, EH], f16, kind="ExternalInput")
    wzT_d = nc.dram_tensor("wzT", [D, EH], f16, kind="ExternalInput")
    convw_d = nc.dram_tensor("convw", [128, M_TILES * K], f32, kind="ExternalInput")
    convb_d = nc.dram_tensor("convb", [128, M_TILES], f32, kind="ExternalInput")
    xpT_d = nc.dram_tensor("xpT", [EH, 96], f16, kind="ExternalInput")
    dtwT_d = nc.dram_tensor("dtwT", [DTR, EH], f32, kind="ExternalInput")
    dtb_d = nc.dram_tensor("dtb", [128, M_TILES], f32, kind="ExternalInput")
    arate_d = nc.dram_tensor("arate", [128, M_TILES * N], f32, kind="ExternalInput")
    dp_d = nc.dram_tensor("dp", [128, M_TILES], f32, kind="ExternalInput")
    woT_d = nc.dram_tensor("woT", [EH, D], f16, kind="ExternalInput")
    mf_d = nc.dram_tensor("mf", [128, 1], f32, kind="ExternalInput")
    mb_d = nc.dram_tensor("mb", [128, 1], f32, kind="ExternalInput")

    dbl_in = nc.dram_tensor("dbl_in", [96, L], f32, kind="Internal")
    dbl_out = nc.dram_tensor("dbl_out", [96, L], f32, kind="Internal")
    bc16_d = nc.dram_tensor("bc16", [32, L], f16, kind="Internal")
    oc_in = nc.dram_tensor("oc_in", [D, L], f32, kind="Internal")
    oc_out = nc.dram_tensor("oc_out", [D, L], f32, kind="Internal")
    out_d = nc.dram_tensor("out_p", [D, L], f32, kind="ExternalOutput")

    with tile.TileContext(nc) as tc:
        with tc.tile_pool(name="const", bufs=1) as cpool, \
             tc.tile_pool(name="res", bufs=1) as rpool:
            convw = cpool.tile([128, M_TILES * K], f32)
            convb = cpool.tile([128, M_TILES], f32)
            dtb = cpool.tile([128, M_TILES], f32)
            arate = cpool.tile([128, M_TILES * N], f32)
            dp = cpool.tile([128, M_TILES], f32)
            mf = cpool.tile([128, 1], f32)
            mb = cpool.tile([128, 1], f32)
            for t_, d_ in ((convw, convw_d), (convb, convb_d), (dtb, dtb_d),
                           (arate, arate_d), (dp, dp_d), (mf, mf_d), (mb, mb_d)):
                nc.sync.dma_start(t_[:], d_[:])

            xc16 = rpool.tile([128, M_TILES * L], f16)
            sz16 = rpool.tile([128, M_TILES * L], f16)
            g16 = rpool.tile([128, M_TILES * L], f16)
            bca = rpool.tile([128, N * L], f16)
            bcc = rpool.tile([128, N * L], f16)
            dblr = rpool.tile([96, L], f32)

            # ---------- Phase A: in_proj matmuls + conv + silu ----------
            with tc.tile_pool(name="pa", bufs=1) as pap, \
                 tc.tile_pool(name="paw", bufs=2) as pwp, \
                 tc.tile_pool(name="pax", bufs=2) as pxp, \
                 tc.tile_pool(name="psA", bufs=2, space="PSUM") as psA:
                xT = pap.tile([128, M_TILES * (3 + L)], f16)
                for kt in range(M_TILES):
                    nc.sync.dma_start(xT[:, kt * (3 + L):(kt + 1) * (3 + L)],
                                      xT_d[kt * 128:(kt + 1) * 128, :])
                for m in range(M_TILES):
                    wxi = pwp.tile([128, M_TILES * 128], f16, tag="wxi")
                    wz = pwp.tile([128, M_TILES * 128], f16, tag="wz")
                    for kt in range(M_TILES):
                        nc.sync.dma_start(wxi[:, kt * 128:(kt + 1) * 128],
                                          wxiT_d[kt * 128:(kt + 1) * 128, m * 128:(m + 1) * 128])
                        nc.sync.dma_start(wz[:, kt * 128:(kt + 1) * 128],
                                          wzT_d[kt * 128:(kt + 1) * 128, m * 128:(m + 1) * 128])
                    ps_xi = psA.tile([128, L], f32, tag="xi")
                    ps_z = psA.tile([128, L], f32, tag="z")
                    for kt in range(M_TILES):
                        xk = xT[:, kt * (3 + L):(kt + 1) * (3 + L)]
                        for h in range(2):
                            nc.tensor.matmul(ps_xi[:, h * 512:(h + 1) * 512],
                                             wxi[:, kt * 128:(kt + 1) * 128],
                                             xk[:, 3 + h * 512: 3 + (h + 1) * 512],
                                             start=(kt == 0), stop=(kt == M_TILES - 1))
                            nc.tensor.matmul(ps_z[:, h * 512:(h + 1) * 512],
                                             wz[:, kt * 128:(kt + 1) * 128],
                                             xk[:, 3 + h * 512: 3 + (h + 1) * 512],
                                             start=(kt == 0), stop=(kt == M_TILES - 1))
                    # conv: xi32 padded copy, then 4-tap chain on DVE
                    xi32 = pxp.tile([128, 3 + L], f32, tag="xi32")
                    nc.vector.memset(xi32[:, 0:3], 0.0)
                    nc.scalar.copy(xi32[:, 3:3 + L], ps_xi[:])
                    cacc = pxp.tile([128, L], f32, tag="cacc")
                    nc.vector.tensor_scalar_mul(cacc[:], xi32[:, 0:L], convw[:, m * K:m * K + 1])
                    for k in range(1, K):
                        nc.vector.scalar_tensor_tensor(
                            cacc[:], xi32[:, k:k + L], convw[:, m * K + k:m * K + k + 1],
                            cacc[:], Alu.mult, Alu.add)
                    nc.scalar.activation(xc16[:, m * L:(m + 1) * L], cacc[:],
                                         Act.Silu, bias=convb[:, m:m + 1])
                    nc.scalar.activation(sz16[:, m * L:(m + 1) * L], ps_z[:], Act.Silu)

            # ---------- Phase B: x_proj partial + AllReduce + broadcasts ----------
            with tc.tile_pool(name="pb", bufs=1) as pbp, \
                 tc.tile_pool(name="pbw", bufs=2) as pbw, \
                 tc.tile_pool(name="psB", bufs=1, space="PSUM") as psB:
                ps_dbl = psB.tile([96, L], f32)
                for m in range(M_TILES):
                    xp = pbw.tile([128, 96], f16, tag="xp")
                    nc.sync.dma_start(xp[:], xpT_d[m * 128:(m + 1) * 128, :])
                    for h in range(2):
                        nc.tensor.matmul(ps_dbl[:, h * 512:(h + 1) * 512], xp[:],
                                         xc16[:, m * L + h * 512: m * L + (h + 1) * 512],
                                         start=(m == 0), stop=(m == M_TILES - 1))
                dbl_sb = pbp.tile([96, L], f32)
                nc.vector.tensor_copy(dbl_sb[:], ps_dbl[:])
                nc.sync.dma_start(dbl_in[:], dbl_sb[:])
                nc.gpsimd.collective_compute(
                    "AllReduce", Alu.add,
                    replica_groups=[[0, 1], [2, 3], [4, 5], [6, 7]],
                    ins=[dbl_in[:]], outs=[dbl_out[:]])
                nc.sync.dma_start(dblr[:], dbl_out[:])
                # B/C rows -> f16 -> DRAM -> partition-replicated planes
                cvt16 = pbp.tile([128, L], f16)
                nc.vector.tensor_copy(cvt16[64:96, :], dblr[64:96, :])
                nc.sync.dma_start(bc16_d[:], cvt16[64:96, :])
                for n in range(N):
                    nc.sync.dma_start(bca[:, n * L:(n + 1) * L],
                                      bc16_d[n:n + 1, :].broadcast_to([128, L]))
                    nc.sync.dma_start(bcc[:, n * L:(n + 1) * L],
                                      bc16_d[N + n:N + n + 1, :].broadcast_to([128, L]))

            # ---------- Phase C: delta, dA planes, scan, y ----------
            with tc.tile_pool(name="pc", bufs=2) as pcp, \
                 tc.tile_pool(name="pc1", bufs=1) as pc1, \
                 tc.tile_pool(name="psC", bufs=2, space="PSUM") as psC:
                bca3 = bca[:].rearrange("p (n l) -> p n l", l=L)
                bcc3 = bcc[:].rearrange("p (n l) -> p n l", l=L)
                for m in range(M_TILES):
                    dtw = pcp.tile([DTR, 128], f32, tag="dtw")
                    nc.sync.dma_start(dtw[:], dtwT_d[:, m * 128:(m + 1) * 128])
                    ps_dt = psC.tile([128, L], f32, tag="dt")
                    for h in range(2):
                        nc.tensor.matmul(ps_dt[:, h * 512:(h + 1) * 512], dtw[:],
                                         dblr[0:DTR, h * 512:(h + 1) * 512],
                                         start=True, stop=True)
                    # softplus(raw) = ln(1 + exp(raw)); Softplus has no act table here
                    delta32 = pcp.tile([128, L], f32, tag="d32")
                    delta16 = pcp.tile([128, L], f16, tag="d16")
                    ee = pcp.tile([128, L], f32, tag="ee")
                    nc.scalar.activation(ee[:], ps_dt[:], Act.Exp, bias=dtb[:, m:m + 1])
                    nc.scalar.activation(delta32[:], ee[:], Act.Ln, bias=1.0)
                    nc.vector.tensor_copy(delta16[:], delta32[:])
                    u16 = pcp.tile([128, L], f16, tag="u16")
                    nc.vector.tensor_mul(u16[:], delta16[:], xc16[:, m * L:(m + 1) * L])
                    yparts = pc1.tile([128, NB * L], f16, tag="yp")
                    for nb in range(NB):
                        dA = pcp.tile([128, NPB * PL], f32, tag="dA")
                        dBu = pcp.tile([128, NPB * PL], f16, tag="dBu")
                        for j in range(NPB):
                            n = nb * NPB + j
                            nc.scalar.activation(dA[:, j * PL:j * PL + L], delta32[:],
                                                 Act.Exp, scale=arate[:, m * N + n:m * N + n + 1])
                        dA3 = dA[:].rearrange("p (n l) -> p n l", l=PL)
                        dBu3 = dBu[:].rearrange("p (n l) -> p n l", l=PL)
                        nc.vector.memset(dA3[:, :, L:PL], 0.0)
                        nc.vector.memset(dBu3[:, :, L:PL], 0.0)
                        nc.vector.tensor_mul(
                            dBu3[:, :, 0:L],
                            u16[:, None, :].broadcast_to([128, NPB, L]),
                            bca3[:, nb * NPB:(nb + 1) * NPB, :])
                        h4 = pcp.tile([128, NPB * PL], f16, tag="h4")
                        nc.vector.tensor_tensor_scan(h4[:], dA[:], dBu[:], 0.0,
                                                     Alu.mult, Alu.add)
                        h43 = h4[:].rearrange("p (n l) -> p n l", l=PL)
                        prod = pcp.tile([128, NPB * L], f16, tag="dBu")
                        prod3 = prod[:].rearrange("p (n l) -> p n l", l=L)
                        nc.vector.tensor_mul(prod3[:], h43[:, :, 0:L],
                                             bcc3[:, nb * NPB:(nb + 1) * NPB, :])
                        nc.vector.tensor_add(yparts[:, nb * L:(nb + 1) * L],
                                             prod[:, 0:L], prod[:, L:2 * L])
                    t4 = pc1.tile([128, 4 * L], f16, tag="t4")
                    nc.vector.tensor_add(t4[:], yparts[:, 0:4 * L], yparts[:, 4 * L:8 * L])
                    t2 = pc1.tile([128, 2 * L], f16, tag="t2")
                    nc.vector.tensor_add(t2[:], t4[:, 0:2 * L], t4[:, 2 * L:4 * L])
                    y16 = pc1.tile([128, L], f16, tag="y16")
                    nc.vector.tensor_add(y16[:], t2[:, 0:L], t2[:, L:2 * L])
                    ys16 = pc1.tile([128, L], f16, tag="ys16")
                    nc.vector.scalar_tensor_tensor(ys16[:], xc16[:, m * L:(m + 1) * L],
                                                   dp[:, m:m + 1], y16[:], Alu.mult, Alu.add)
                    nc.vector.tensor_mul(g16[:, m * L:(m + 1) * L], ys16[:],
                                         sz16[:, m * L:(m + 1) * L])

            # ---------- Phase D: out_proj + flip-combine + AllReduce ----------
            with tc.tile_pool(name="pd", bufs=2) as pdp, \
                 tc.tile_pool(name="psD", bufs=2, space="PSUM") as psD:
                for dm in range(M_TILES):
                    wo = pdp.tile([128, M_TILES * 128], f16, tag="wo")
                    for m in range(M_TILES):
                        nc.sync.dma_start(wo[:, m * 128:(m + 1) * 128],
                                          woT_d[m * 128:(m + 1) * 128, dm * 128:(dm + 1) * 128])
                    ps_o = psD.tile([128, L], f32, tag="o")
                    for m in range(M_TILES):
                        for h in range(2):
                            nc.tensor.matmul(ps_o[:, h * 512:(h + 1) * 512],
                                             wo[:, m * 128:(m + 1) * 128],
                                             g16[:, m * L + h * 512: m * L + (h + 1) * 512],
                                             start=(m == 0), stop=(m == M_TILES - 1))
                    o32 = pdp.tile([128, L], f32, tag="o32")
                    nc.vector.tensor_copy(o32[:], ps_o[:])
                    t1 = pdp.tile([128, L], f32, tag="t1")
                    nc.vector.tensor_scalar_mul(t1[:], o32[:], mf[:, 0:1])
                    ocs = pdp.tile([128, L], f32, tag="ocs")
                    nc.vector.scalar_tensor_tensor(ocs[:], o32[:, ::-1], mb[:, 0:1],
                                                   t1[:], Alu.mult, Alu.add)
                    nc.sync.dma_start(oc_in[dm * 128:(dm + 1) * 128, :], ocs[:])
                nc.gpsimd.collective_compute(
                    "AllReduce", Alu.add,
                    replica_groups=[[0, 1, 2, 3], [4, 5, 6, 7]],
                    ins=[oc_in[:]], outs=[oc_out[:]])
                with tc.tile_pool(name="po", bufs=2) as pop:
                    for dm in range(M_TILES):
                        ot = pop.tile([128, L], f32, tag="ot")
                        nc.sync.dma_start(ot[:], oc_out[dm * 128:(dm + 1) * 128, :])
                        nc.sync.dma_start(out_d[dm * 128:(dm + 1) * 128, :], ot[:])

    nc.compile()
    return nc


def _host_prep(inputs):
    """Build the 8 per-core input maps from the full problem inputs."""
    x = np.asarray(inputs["x"], np.float32)
    merge_w = np.asarray(inputs["merge_w"], np.float32)
    in_maps = []
    for b in range(B):
        for di, pre in enumerate(("fwd", "bwd")):
            p = {k: np.asarray(inputs[f"{pre}_{k}"], np.float32)
                 for k in ("in_proj", "conv_w", "conv_b", "x_proj", "dt_w",
                           "dt_b", "A_log", "D", "out_proj")}
            xb = x[b]
            if di == 1:
                xb = xb[::-1]
            xTp = np.concatenate([np.zeros((D, 3), np.float32), xb.T], axis=1)
            A = -np.exp(p["A_log"])                       # (E, N)
            W = merge_w[:, di * D:(di + 1) * D] @ p["out_proj"]   # (D, E)
            for half in range(2):
                sl = slice(half * EH, (half + 1) * EH)
                wxiT = p["in_proj"][:E][sl].T             # (D, EH)
                wzT = p["in_proj"][E:][sl].T
                convw = p["conv_w"][sl].reshape(M_TILES, 128, K).transpose(1, 0, 2).reshape(128, M_TILES * K)
                convb = p["conv_b"][sl].reshape(M_TILES, 128).T
                xpT = p["x_proj"][:, sl].T                # (EH, 96)
                dtwT = p["dt_w"][sl].T                    # (DTR, EH)
                dtb = p["dt_b"][sl].reshape(M_TILES, 128).T
                arate = A[sl].reshape(M_TILES, 128, N).transpose(1, 0, 2).reshape(128, M_TILES * N)
                dpv = p["D"][sl].reshape(M_TILES, 128).T
                woT = W[:, sl].T                          # (EH, D)
                fwd = (di == 0)
                in_maps.append({
                    "xT": xTp.astype(np.float16),
                    "wxiT": wxiT.astype(np.float16),
                    "wzT": wzT.astype(np.float16),
                    "convw": np.ascontiguousarray(convw, np.float32),
                    "convb": np.ascontiguousarray(convb, np.float32),
                    "xpT": xpT.astype(np.float16),
                    "dtwT": np.ascontiguousarray(dtwT, np.float32),
                    "dtb": np.ascontiguousarray(dtb, np.float32),
                    "arate": np.ascontiguousarray(arate, np.float32),
                    "dp": np.ascontiguousarray(dpv, np.float32),
                    "woT": woT.astype(np.float16),
                    "mf": np.full((128, 1), 1.0 if fwd else 0.0, np.float32),
                    "mb": np.full((128, 1), 0.0 if fwd else 1.0, np.float32),
                })
    return in_maps


def kernel(**inputs):
    from concourse.bass_utils import run_bass_kernel_spmd
    if "nc" not in _nc_cache:
        _nc_cache["nc"] = _build_nc()
    nc = _nc_cache["nc"]
    in_maps = _host_prep(inputs)
    res = run_bass_kernel_spmd(nc, in_maps, core_ids=list(range(8)))
    _nc_cache["last_results"] = res
    out = np.stack([res.results[0]["out_p"].T, res.results[4]["out_p"].T])
    return out.astype(np.float32)


# revision 14
# speedup vs baseline: 1.0983x; 1.0983x over previous
"""BiMamba (fwd+bwd Mamba + merge) Trainium2 Bass kernel.

Sharding (8 cores): core = batch*4 + dir*2 + e_half.
Each core computes one (batch, direction) pair over 1024 of the 2048 d_inner
channels, in e-partition layout [e_p=128 x 8 tiles, t_free=1024].
bwd cores operate entirely in flipped time (host pre-flips x); the final
out_proj partial is un-flipped via a data-driven mask combine, then a 4-core
AllReduce produces the full (d, t) output on every core of the batch group.

Self-contained: hardcodes B=2, L=1024, D=1024, E=2048 (1024/core), N=16,
dt_rank=64, d_conv=4.
"""
import numpy as np

B, L, D = 2, 1024, 1024
E = 2048
EH = 1024            # channels per core (half of E)
N = 16
DTR = 64
K = 4                # d_conv
M_TILES = 8          # e-tiles per core
NB = 8               # n-plane batches
NPB = 2              # planes per batch
PL = L + 2           # plane stride with 2-col zero gap for the batched scan

_nc_cache = {}


def _build_nc():
    import concourse.bacc as bacc
    import concourse.mybir as mybir
    from concourse import tile

    f32, f16 = mybir.dt.float32, mybir.dt.float16
    Alu = mybir.AluOpType
    Act = mybir.ActivationFunctionType

    nc = bacc.Bacc("TRN2", target_bir_lowering=False, debug=False, num_devices=8)

    # ---- DRAM I/O ----
    xT_d = nc.dram_tensor("xT", [D, 3 + L], f16, kind="ExternalInput")
    # pre-tiled: [p, m*1024 + kt*128 + e']  (one DMA per m-slab)
    wxiT_d = nc.dram_tensor("wxiT", [128, M_TILES * EH], f16, kind="ExternalInput")
    wzT_d = nc.dram_tensor("wzT", [128, M_TILES * EH], f16, kind="ExternalInput")
    convw_d = nc.dram_tensor("convw", [128, M_TILES * K], f32, kind="ExternalInput")
    convb_d = nc.dram_tensor("convb", [128, M_TILES], f32, kind="ExternalInput")
    xpT_d = nc.dram_tensor("xpT", [EH, 96], f16, kind="ExternalInput")
    dtwT_d = nc.dram_tensor("dtwT", [DTR, EH], f32, kind="ExternalInput")
    dtb_d = nc.dram_tensor("dtb", [128, M_TILES], f32, kind="ExternalInput")
    arate_d = nc.dram_tensor("arate", [128, M_TILES * N], f32, kind="ExternalInput")
    dp_d = nc.dram_tensor("dp", [128, M_TILES], f32, kind="ExternalInput")
    # pre-tiled: [p, dm*1024 + m*128 + d']
    woT_d = nc.dram_tensor("woT", [128, M_TILES * D], f16, kind="ExternalInput")
    mf_d = nc.dram_tensor("mf", [128, 1], f32, kind="ExternalInput")
    mb_d = nc.dram_tensor("mb", [128, 1], f32, kind="ExternalInput")

    dbl_in = nc.dram_tensor("dbl_in", [96, L], f32, kind="Internal")
    dbl_out = nc.dram_tensor("dbl_out", [96, L], f32, kind="Internal")
    bc16_d = nc.dram_tensor("bc16", [32, L], f16, kind="Internal")
    oc_in = nc.dram_tensor("oc_in", [D, L], f16, kind="Internal")
    oc_out = nc.dram_tensor("oc_out", [D, L], f16, kind="Internal")
    out_d = nc.dram_tensor("out_p", [D, L], f16, kind="ExternalOutput")

    with tile.TileContext(nc) as tc:
        with tc.tile_pool(name="const", bufs=1) as cpool, \
             tc.tile_pool(name="res", bufs=1) as rpool:
            convw = cpool.tile([128, M_TILES * K], f32)
            convb = cpool.tile([128, M_TILES], f32)
            dtb = cpool.tile([128, M_TILES], f32)
            arate = cpool.tile([128, M_TILES * N], f32)
            dp = cpool.tile([128, M_TILES], f32)
            mf = cpool.tile([128, 1], f32)
            mb = cpool.tile([128, 1], f32)
            for t_, d_ in ((convw, convw_d), (convb, convb_d), (dtb, dtb_d),
                           (arate, arate_d), (dp, dp_d), (mf, mf_d), (mb, mb_d)):
                nc.sync.dma_start(t_[:], d_[:])

            xc16 = rpool.tile([128, M_TILES * L], f16)
            sz16 = rpool.tile([128, M_TILES * L], f16)
            g16 = rpool.tile([128, M_TILES * L], f16)
            bca = rpool.tile([128, N * L], f16)
            bcc = rpool.tile([128, N * L], f16)
            dblr = rpool.tile([96, L], f32)

            # ---------- Phase A: in_proj matmuls + conv + silu ----------
            with tc.tile_pool(name="pa", bufs=1) as pap, \
                 tc.tile_pool(name="paw", bufs=2) as pwp, \
                 tc.tile_pool(name="pax", bufs=2) as pxp, \
                 tc.tile_pool(name="psA", bufs=2, space="PSUM") as psA:
                xT = pap.tile([128, M_TILES * (3 + L)], f16)
                for kt in range(M_TILES):
                    nc.sync.dma_start(xT[:, kt * (3 + L):(kt + 1) * (3 + L)],
                                      xT_d[kt * 128:(kt + 1) * 128, :])
                for m in range(M_TILES):
                    wxi = pwp.tile([128, M_TILES * 128], f16, tag="wxi")
                    wz = pwp.tile([128, M_TILES * 128], f16, tag="wz")
                    nc.sync.dma_start(wxi[:], wxiT_d[:, m * EH:(m + 1) * EH])
                    nc.sync.dma_start(wz[:], wzT_d[:, m * EH:(m + 1) * EH])
                    ps_xi = psA.tile([128, L], f32, tag="xi")
                    ps_z = psA.tile([128, L], f32, tag="z")
                    for kt in range(M_TILES):
                        xk = xT[:, kt * (3 + L):(kt + 1) * (3 + L)]
                        for h in range(2):
                            nc.tensor.matmul(ps_xi[:, h * 512:(h + 1) * 512],
                                             wxi[:, kt * 128:(kt + 1) * 128],
                                             xk[:, 3 + h * 512: 3 + (h + 1) * 512],
                                             start=(kt == 0), stop=(kt == M_TILES - 1))
                            nc.tensor.matmul(ps_z[:, h * 512:(h + 1) * 512],
                                             wz[:, kt * 128:(kt + 1) * 128],
                                             xk[:, 3 + h * 512: 3 + (h + 1) * 512],
                                             start=(kt == 0), stop=(kt == M_TILES - 1))
                    # conv: xi32 padded copy, then 4-tap chain on DVE
                    xi32 = pxp.tile([128, 3 + L], f32, tag="xi32")
                    nc.vector.memset(xi32[:, 0:3], 0.0)
                    nc.scalar.copy(xi32[:, 3:3 + L], ps_xi[:])
                    cacc = pxp.tile([128, L], f32, tag="cacc")
                    nc.vector.tensor_scalar_mul(cacc[:], xi32[:, 0:L], convw[:, m * K:m * K + 1])
                    for k in range(1, K):
                        nc.vector.scalar_tensor_tensor(
                            cacc[:], xi32[:, k:k + L], convw[:, m * K + k:m * K + k + 1],
                            cacc[:], Alu.mult, Alu.add)
                    nc.scalar.activation(xc16[:, m * L:(m + 1) * L], cacc[:],
                                         Act.Silu, bias=convb[:, m:m + 1])
                    nc.scalar.activation(sz16[:, m * L:(m + 1) * L], ps_z[:], Act.Silu)

            # ---------- Phase B: x_proj partial + AllReduce + broadcasts ----------
            with tc.tile_pool(name="pb", bufs=1) as pbp, \
                 tc.tile_pool(name="pbw", bufs=2) as pbw, \
                 tc.tile_pool(name="psB", bufs=1, space="PSUM") as psB:
                ps_dbl = psB.tile([96, L], f32)
                for m in range(M_TILES):
                    xp = pbw.tile([128, 96], f16, tag="xp")
                    nc.sync.dma_start(xp[:], xpT_d[m * 128:(m + 1) * 128, :])
                    for h in range(2):
                        nc.tensor.matmul(ps_dbl[:, h * 512:(h + 1) * 512], xp[:],
                                         xc16[:, m * L + h * 512: m * L + (h + 1) * 512],
                                         start=(m == 0), stop=(m == M_TILES - 1))
                dbl_sb = pbp.tile([96, L], f32)
                nc.vector.tensor_copy(dbl_sb[:], ps_dbl[:])
                nc.sync.dma_start(dbl_in[:], dbl_sb[:])
                nc.gpsimd.collective_compute(
                    "AllReduce", Alu.add,
                    replica_groups=[[0, 1], [2, 3], [4, 5], [6, 7]],
                    ins=[dbl_in[:]], outs=[dbl_out[:]])
                nc.sync.dma_start(dblr[:], dbl_out[:])
                # B/C rows -> f16 -> DRAM -> partition-replicated planes
                cvt16 = pbp.tile([128, L], f16)
                nc.vector.tensor_copy(cvt16[64:96, :], dblr[64:96, :])
                nc.sync.dma_start(bc16_d[:], cvt16[64:96, :])
                for n in range(N):
                    nc.sync.dma_start(bca[:, n * L:(n + 1) * L],
                                      bc16_d[n:n + 1, :].broadcast_to([128, L]))
                    nc.sync.dma_start(bcc[:, n * L:(n + 1) * L],
                                      bc16_d[N + n:N + n + 1, :].broadcast_to([128, L]))

            # ---------- Phase C: delta, dA planes, scan, y ----------
            with tc.tile_pool(name="pc", bufs=2) as pcp, \
                 tc.tile_pool(name="pc1", bufs=1) as pc1, \
                 tc.tile_pool(name="psC", bufs=2, space="PSUM") as psC:
                bca3 = bca[:].rearrange("p (n l) -> p n l", l=L)
                bcc3 = bcc[:].rearrange("p (n l) -> p n l", l=L)
                for m in range(M_TILES):
                    dtw = pcp.tile([DTR, 128], f32, tag="dtw")
                    nc.sync.dma_start(dtw[:], dtwT_d[:, m * 128:(m + 1) * 128])
                    ps_dt = psC.tile([128, L], f32, tag="dt")
                    for h in range(2):
                        nc.tensor.matmul(ps_dt[:, h * 512:(h + 1) * 512], dtw[:],
                                         dblr[0:DTR, h * 512:(h + 1) * 512],
                                         start=True, stop=True)
                    # softplus(raw) = ln(1 + exp(raw)); Softplus has no act table here
                    delta32 = pcp.tile([128, L], f32, tag="d32")
                    delta16 = pcp.tile([128, L], f16, tag="d16")
                    ee = pcp.tile([128, L], f32, tag="ee")
                    nc.scalar.activation(ee[:], ps_dt[:], Act.Exp, bias=dtb[:, m:m + 1])
                    nc.scalar.activation(delta32[:], ee[:], Act.Ln, bias=1.0)
                    nc.vector.tensor_copy(delta16[:], delta32[:])
                    u16 = pcp.tile([128, L], f16, tag="u16")
                    nc.vector.tensor_mul(u16[:], delta16[:], xc16[:, m * L:(m + 1) * L])
                    yparts = pc1.tile([128, NB * L], f16, tag="yp")
                    for nb in range(NB):
                        dA = pcp.tile([128, NPB * PL], f32, tag="dA")
                        dBu = pcp.tile([128, NPB * PL], f16, tag="dBu")
                        for j in range(NPB):
                            n = nb * NPB + j
                            nc.scalar.activation(dA[:, j * PL:j * PL + L], delta32[:],
                                                 Act.Exp, scale=arate[:, m * N + n:m * N + n + 1])
                        dA3 = dA[:].rearrange("p (n l) -> p n l", l=PL)
                        dBu3 = dBu[:].rearrange("p (n l) -> p n l", l=PL)
                        nc.vector.memset(dA3[:, :, L:PL], 0.0)
                        nc.vector.memset(dBu3[:, :, L:PL], 0.0)
                        nc.vector.tensor_mul(
                            dBu3[:, :, 0:L],
                            u16[:, None, :].broadcast_to([128, NPB, L]),
                            bca3[:, nb * NPB:(nb + 1) * NPB, :])
                        h4 = pcp.tile([128, NPB * PL], f16, tag="h4")
                        nc.vector.tensor_tensor_scan(h4[:], dA[:], dBu[:], 0.0,
                                                     Alu.mult, Alu.add)
                        h43 = h4[:].rearrange("p (n l) -> p n l", l=PL)
                        prod = pcp.tile([128, NPB * L], f16, tag="dBu")
                        prod3 = prod[:].rearrange("p (n l) -> p n l", l=L)
                        nc.vector.tensor_mul(prod3[:], h43[:, :, 0:L],
                                             bcc3[:, nb * NPB:(nb + 1) * NPB, :])
                        nc.vector.tensor_add(yparts[:, nb * L:(nb + 1) * L],
                                             prod[:, 0:L], prod[:, L:2 * L])
                    t4 = pc1.tile([128, 4 * L], f16, tag="t4")
                    nc.vector.tensor_add(t4[:], yparts[:, 0:4 * L], yparts[:, 4 * L:8 * L])
                    t2 = pc1.tile([128, 2 * L], f16, tag="t2")
                    nc.vector.tensor_add(t2[:], t4[:, 0:2 * L], t4[:, 2 * L:4 * L])
                    y16 = pc1.tile([128, L], f16, tag="y16")
                    nc.vector.tensor_add(y16[:], t2[:, 0:L], t2[:, L:2 * L])
                    ys16 = pc1.tile([128, L], f16, tag="ys16")
                    nc.vector.scalar_tensor_tensor(ys16[:], xc16[:, m * L:(m + 1) * L],
                                                   dp[:, m:m + 1], y16[:], Alu.mult, Alu.add)
                    nc.vector.tensor_mul(g16[:, m * L:(m + 1) * L], ys16[:],
                                         sz16[:, m * L:(m + 1) * L])

            # ---------- Phase D: out_proj + flip-combine + AllReduce ----------
            with tc.tile_pool(name="pd", bufs=2) as pdp, \
                 tc.tile_pool(name="psD", bufs=2, space="PSUM") as psD:
                for dm in range(M_TILES):
                    wo = pdp.tile([128, M_TILES * 128], f16, tag="wo")
                    nc.sync.dma_start(wo[:], woT_d[:, dm * D:(dm + 1) * D])
                    ps_o = psD.tile([128, L], f32, tag="o")
                    for m in range(M_TILES):
                        for h in range(2):
                            nc.tensor.matmul(ps_o[:, h * 512:(h + 1) * 512],
                                             wo[:, m * 128:(m + 1) * 128],
                                             g16[:, m * L + h * 512: m * L + (h + 1) * 512],
                                             start=(m == 0), stop=(m == M_TILES - 1))
                    o32 = pdp.tile([128, L], f32, tag="o32")
                    nc.vector.tensor_copy(o32[:], ps_o[:])
                    t1 = pdp.tile([128, L], f16, tag="t1")
                    nc.vector.tensor_scalar_mul(t1[:], o32[:], mf[:, 0:1])
                    ocs = pdp.tile([128, L], f16, tag="ocs")
                    nc.vector.scalar_tensor_tensor(ocs[:], o32[:, ::-1], mb[:, 0:1],
                                                   t1[:], Alu.mult, Alu.add)
                    nc.sync.dma_start(oc_in[dm * 128:(dm + 1) * 128, :], ocs[:])
                    if dm == 3:
                        nc.gpsimd.collective_compute(
                            "AllReduce", Alu.add,
                            replica_groups=[[0, 1, 2, 3], [4, 5, 6, 7]],
                            ins=[oc_in[0:512, :]], outs=[oc_out[0:512, :]])
                        nc.sync.dma_start(out_d[0:512, :], oc_out[0:512, :])
                nc.gpsimd.collective_compute(
                    "AllReduce", Alu.add,
                    replica_groups=[[0, 1, 2, 3], [4, 5, 6, 7]],
                    ins=[oc_in[512:1024, :]], outs=[oc_out[512:1024, :]])
                nc.sync.dma_start(out_d[512:1024, :], oc_out[512:1024, :])

    nc.compile()
    return nc


def _host_prep(inputs):
    """Build the 8 per-core input maps from the full problem inputs."""
    x = np.asarray(inputs["x"], np.float32)
    merge_w = np.asarray(inputs["merge_w"], np.float32)
    in_maps = []
    for b in range(B):
        for di, pre in enumerate(("fwd", "bwd")):
            p = {k: np.asarray(inputs[f"{pre}_{k}"], np.float32)
                 for k in ("in_proj", "conv_w", "conv_b", "x_proj", "dt_w",
                           "dt_b", "A_log", "D", "out_proj")}
            xb = x[b]
            if di == 1:
                xb = xb[::-1]
            xTp = np.concatenate([np.zeros((D, 3), np.float32), xb.T], axis=1)
            A = -np.exp(p["A_log"])                       # (E, N)
            W = merge_w[:, di * D:(di + 1) * D] @ p["out_proj"]   # (D, E)
            def pack_lhsT(wT):
                # (D, EH) -> [p, m*1024 + kt*128 + e']
                return np.ascontiguousarray(
                    wT.reshape(M_TILES, 128, M_TILES, 128).transpose(1, 2, 0, 3)
                    .reshape(128, M_TILES * EH))

            for half in range(2):
                sl = slice(half * EH, (half + 1) * EH)
                wxiT = pack_lhsT(p["in_proj"][:E][sl].T)
                wzT = pack_lhsT(p["in_proj"][E:][sl].T)
                convw = p["conv_w"][sl].reshape(M_TILES, 128, K).transpose(1, 0, 2).reshape(128, M_TILES * K)
                convb = p["conv_b"][sl].reshape(M_TILES, 128).T
                xpT = p["x_proj"][:, sl].T                # (EH, 96)
                dtwT = p["dt_w"][sl].T                    # (DTR, EH)
                dtb = p["dt_b"][sl].reshape(M_TILES, 128).T
                arate = A[sl].reshape(M_TILES, 128, N).transpose(1, 0, 2).reshape(128, M_TILES * N)
                dpv = p["D"][sl].reshape(M_TILES, 128).T
                woT = pack_lhsT(W[:, sl].T)               # (EH, D) pre-tiled
                fwd = (di == 0)
                in_maps.append({
                    "xT": xTp.astype(np.float16),
                    "wxiT": wxiT.astype(np.float16),
                    "wzT": wzT.astype(np.float16),
                    "convw": np.ascontiguousarray(convw, np.float32),
                    "convb": np.ascontiguousarray(convb, np.float32),
                    "xpT": xpT.astype(np.float16),
                    "dtwT": np.ascontiguousarray(dtwT, np.float32),
                    "dtb": np.ascontiguousarray(dtb, np.float32),
                    "arate": np.ascontiguousarray(arate, np.float32),
                    "dp": np.ascontiguousarray(dpv, np.float32),
                    "woT": woT.astype(np.float16),
                    "mf": np.full((128, 1), 1.0 if fwd else 0.0, np.float32),
                    "mb": np.full((128, 1), 0.0 if fwd else 1.0, np.float32),
                })
    return in_maps


def kernel(**inputs):
    from concourse.bass_utils import run_bass_kernel_spmd
    if "nc" not in _nc_cache:
        _nc_cache["nc"] = _build_nc()
    nc = _nc_cache["nc"]
    in_maps = _host_prep(inputs)
    res = run_bass_kernel_spmd(nc, in_maps, core_ids=list(range(8)))
    _nc_cache["last_results"] = res
    out = np.stack([res.results[0]["out_p"].T, res.results[4]["out_p"].T])
    return out.astype(np.float32)


# revision 23
# speedup vs baseline: 1.1379x; 1.0361x over previous
"""BiMamba (fwd+bwd Mamba + merge) Trainium2 Bass kernel.

Sharding (8 cores): core = batch*4 + dir*2 + e_half.
Each core computes one (batch, direction) pair over 1024 of the 2048 d_inner
channels, in e-partition layout [e_p=128 x 8 tiles, t_free=1024].
bwd cores operate entirely in flipped time (host pre-flips x); the final
out_proj partial is un-flipped via a data-driven mask combine, then a 4-core
AllReduce produces the full (d, t) output on every core of the batch group.

Self-contained: hardcodes B=2, L=1024, D=1024, E=2048 (1024/core), N=16,
dt_rank=64, d_conv=4.
"""
import numpy as np

B, L, D = 2, 1024, 1024
E = 2048
EH = 1024            # channels per core (half of E)
N = 16
DTR = 64
K = 4                # d_conv
M_TILES = 8          # e-tiles per core
NB = 8               # n-plane batches
NPB = 2              # planes per batch
PL = L + 2           # plane stride with 2-col zero gap for the batched scan

_nc_cache = {}


def _build_nc():
    import concourse.bacc as bacc
    import concourse.mybir as mybir
    from concourse import tile

    f32, f16 = mybir.dt.float32, mybir.dt.float16
    Alu = mybir.AluOpType
    Act = mybir.ActivationFunctionType

    nc = bacc.Bacc("TRN2", target_bir_lowering=False, debug=False, num_devices=8)

    # ---- DRAM I/O ----
    xT_d = nc.dram_tensor("xT", [D, 3 + L], f16, kind="ExternalInput")
    # pre-tiled: [p, m*1024 + kt*128 + e']  (one DMA per m-slab)
    wxiT_d = nc.dram_tensor("wxiT", [128, M_TILES * EH], f16, kind="ExternalInput")
    wzT_d = nc.dram_tensor("wzT", [128, M_TILES * EH], f16, kind="ExternalInput")
    convw_d = nc.dram_tensor("convw", [128, M_TILES * K], f32, kind="ExternalInput")
    convb_d = nc.dram_tensor("convb", [128, M_TILES], f32, kind="ExternalInput")
    xpT_d = nc.dram_tensor("xpT", [EH, 96], f16, kind="ExternalInput")
    dtwT_d = nc.dram_tensor("dtwT", [DTR, EH], f32, kind="ExternalInput")
    dtb_d = nc.dram_tensor("dtb", [128, M_TILES], f32, kind="ExternalInput")
    arate_d = nc.dram_tensor("arate", [128, M_TILES * N], f32, kind="ExternalInput")
    dp_d = nc.dram_tensor("dp", [128, M_TILES], f32, kind="ExternalInput")
    # pre-tiled: [p, dm*1024 + m*128 + d']
    woT_d = nc.dram_tensor("woT", [128, M_TILES * D], f16, kind="ExternalInput")
    mf_d = nc.dram_tensor("mf", [128, 1], f32, kind="ExternalInput")
    mb_d = nc.dram_tensor("mb", [128, 1], f32, kind="ExternalInput")

    dbl_in = nc.dram_tensor("dbl_in", [64, L], f32, kind="Internal")
    dbl_out = nc.dram_tensor("dbl_out", [64, L], f32, kind="Internal")
    bc16_in = nc.dram_tensor("bc16_in", [32, L], f16, kind="Internal")
    bc16_d = nc.dram_tensor("bc16", [32, L], f16, kind="Internal")
    oc_in = nc.dram_tensor("oc_in", [D, L], f16, kind="Internal")
    oc_out = nc.dram_tensor("oc_out", [D, L], f16, kind="Internal")
    out_d = nc.dram_tensor("out_p", [D, L], f16, kind="ExternalOutput")

    with tile.TileContext(nc) as tc:
        with tc.tile_pool(name="const", bufs=1) as cpool, \
             tc.tile_pool(name="res", bufs=1) as rpool:
            convw = cpool.tile([128, M_TILES * K], f32)
            convb = cpool.tile([128, M_TILES], f32)
            dtb = cpool.tile([128, M_TILES], f32)
            arate = cpool.tile([128, M_TILES * N], f32)
            dp = cpool.tile([128, M_TILES], f32)
            mf = cpool.tile([128, 1], f32)
            mb = cpool.tile([128, 1], f32)
            for t_, d_ in ((convw, convw_d), (convb, convb_d), (dtb, dtb_d),
                           (arate, arate_d), (dp, dp_d), (mf, mf_d), (mb, mb_d)):
                nc.sync.dma_start(t_[:], d_[:])

            xc16 = rpool.tile([128, M_TILES * L], f16)
            sz16 = rpool.tile([128, M_TILES * L], f16)
            g16 = rpool.tile([128, M_TILES * L], f16)
            bca = rpool.tile([128, N * L], f16)
            bcc = rpool.tile([128, N * L], f16)
            dblr = rpool.tile([64, L], f32)

            # ---------- Phase A: in_proj matmuls + conv + silu ----------
            with tc.tile_pool(name="pa", bufs=1) as pap, \
                 tc.tile_pool(name="paw", bufs=2) as pwp, \
                 tc.tile_pool(name="pax", bufs=2) as pxp, \
                 tc.tile_pool(name="psA", bufs=2, space="PSUM") as psA:
                xT = pap.tile([128, M_TILES * (3 + L)], f16)
                for kt in range(M_TILES):
                    nc.sync.dma_start(xT[:, kt * (3 + L):(kt + 1) * (3 + L)],
                                      xT_d[kt * 128:(kt + 1) * 128, :])
                for m in range(M_TILES):
                    wxi = pwp.tile([128, M_TILES * 128], f16, tag="wxi")
                    wz = pwp.tile([128, M_TILES * 128], f16, tag="wz")
                    nc.sync.dma_start(wxi[:], wxiT_d[:, m * EH:(m + 1) * EH])
                    nc.sync.dma_start(wz[:], wzT_d[:, m * EH:(m + 1) * EH])
                    ps_xi = psA.tile([128, L], f32, tag="xi")
                    ps_z = psA.tile([128, L], f32, tag="z")
                    for kt in range(M_TILES):
                        xk = xT[:, kt * (3 + L):(kt + 1) * (3 + L)]
                        for h in range(2):
                            nc.tensor.matmul(ps_xi[:, h * 512:(h + 1) * 512],
                                             wxi[:, kt * 128:(kt + 1) * 128],
                                             xk[:, 3 + h * 512: 3 + (h + 1) * 512],
                                             start=(kt == 0), stop=(kt == M_TILES - 1))
                            nc.tensor.matmul(ps_z[:, h * 512:(h + 1) * 512],
                                             wz[:, kt * 128:(kt + 1) * 128],
                                             xk[:, 3 + h * 512: 3 + (h + 1) * 512],
                                             start=(kt == 0), stop=(kt == M_TILES - 1))
                    # conv: xi32 padded copy, then 4-tap chain on DVE
                    xi32 = pxp.tile([128, 3 + L], f32, tag="xi32")
                    nc.vector.memset(xi32[:, 0:3], 0.0)
                    nc.scalar.copy(xi32[:, 3:3 + L], ps_xi[:])
                    cacc = pxp.tile([128, L], f32, tag="cacc")
                    nc.vector.tensor_scalar_mul(cacc[:], xi32[:, 0:L], convw[:, m * K:m * K + 1])
                    for k in range(1, K):
                        nc.vector.scalar_tensor_tensor(
                            cacc[:], xi32[:, k:k + L], convw[:, m * K + k:m * K + k + 1],
                            cacc[:], Alu.mult, Alu.add)
                    nc.scalar.activation(xc16[:, m * L:(m + 1) * L], cacc[:],
                                         Act.Silu, bias=convb[:, m:m + 1])
                    nc.scalar.activation(sz16[:, m * L:(m + 1) * L], ps_z[:], Act.Silu)

            # ---------- Phase B: x_proj partial + AllReduce + broadcasts ----------
            with tc.tile_pool(name="pb", bufs=1) as pbp, \
                 tc.tile_pool(name="pbw", bufs=2) as pbw, \
                 tc.tile_pool(name="psB", bufs=1, space="PSUM") as psB:
                ps_dbl = psB.tile([96, L], f32)
                for m in range(M_TILES):
                    xp = pbw.tile([128, 96], f16, tag="xp")
                    nc.sync.dma_start(xp[:], xpT_d[m * 128:(m + 1) * 128, :])
                    for h in range(2):
                        nc.tensor.matmul(ps_dbl[:, h * 512:(h + 1) * 512], xp[:],
                                         xc16[:, m * L + h * 512: m * L + (h + 1) * 512],
                                         start=(m == 0), stop=(m == M_TILES - 1))
                # split: dt rows AllReduce in f32; B/C rows in f16 (feeds planes)
                dbl_sb = pbp.tile([64, L], f32)
                nc.vector.tensor_copy(dbl_sb[:], ps_dbl[0:64, :])
                nc.sync.dma_start(dbl_in[:], dbl_sb[:])
                cvt16 = pbp.tile([128, L], f16)
                nc.vector.tensor_copy(cvt16[64:96, :], ps_dbl[64:96, :])
                nc.scalar.dma_start(bc16_in[:], cvt16[64:96, :])
                nc.gpsimd.collective_compute(
                    "AllReduce", Alu.add,
                    replica_groups=[[0, 1], [2, 3], [4, 5], [6, 7]],
                    ins=[bc16_in[:]], outs=[bc16_d[:]])
                nc.gpsimd.collective_compute(
                    "AllReduce", Alu.add,
                    replica_groups=[[0, 1], [2, 3], [4, 5], [6, 7]],
                    ins=[dbl_in[:]], outs=[dbl_out[:]])
                nc.sync.dma_start(dblr[0:64, :], dbl_out[:])
                engs = [nc.sync, nc.scalar, nc.gpsimd]
                for n in range(N):
                    engs[n % 3].dma_start(bca[:, n * L:(n + 1) * L],
                                          bc16_d[n:n + 1, :].broadcast_to([128, L]))
                    engs[(n + 1) % 3].dma_start(bcc[:, n * L:(n + 1) * L],
                                                bc16_d[N + n:N + n + 1, :].broadcast_to([128, L]))

            # ---------- Phase C: delta, dA planes, scan, y ----------
            with tc.tile_pool(name="pc", bufs=2) as pcp, \
                 tc.tile_pool(name="pc1", bufs=1) as pc1, \
                 tc.tile_pool(name="psC", bufs=2, space="PSUM") as psC:
                bca3 = bca[:].rearrange("p (n l) -> p n l", l=L)
                bcc3 = bcc[:].rearrange("p (n l) -> p n l", l=L)
                for m in range(M_TILES):
                    dtw = pcp.tile([DTR, 128], f32, tag="dtw")
                    nc.sync.dma_start(dtw[:], dtwT_d[:, m * 128:(m + 1) * 128])
                    ps_dt = psC.tile([128, L], f32, tag="dt")
                    for h in range(2):
                        nc.tensor.matmul(ps_dt[:, h * 512:(h + 1) * 512], dtw[:],
                                         dblr[0:DTR, h * 512:(h + 1) * 512],
                                         start=True, stop=True)
                    # softplus(raw) = ln(1 + exp(raw)); Softplus has no act table here
                    delta32 = pcp.tile([128, L], f32, tag="d32")
                    delta16 = pcp.tile([128, L], f16, tag="d16")
                    ee = pcp.tile([128, L], f32, tag="ee")
                    nc.scalar.activation(ee[:], ps_dt[:], Act.Exp, bias=dtb[:, m:m + 1])
                    nc.scalar.activation(delta32[:], ee[:], Act.Ln, bias=1.0)
                    nc.vector.tensor_copy(delta16[:], delta32[:])
                    u16 = pcp.tile([128, L], f16, tag="u16")
                    nc.vector.tensor_mul(u16[:], delta16[:], xc16[:, m * L:(m + 1) * L])
                    yparts = pc1.tile([128, NB * L], f16, tag="yp")
                    for nb in range(NB):
                        dA = pcp.tile([128, NPB * PL], f32, tag="dA")
                        dBu = pcp.tile([128, NPB * PL], f16, tag="dBu")
                        for j in range(NPB):
                            n = nb * NPB + j
                            nc.scalar.activation(dA[:, j * PL:j * PL + L], delta32[:],
                                                 Act.Exp, scale=arate[:, m * N + n:m * N + n + 1])
                        dA3 = dA[:].rearrange("p (n l) -> p n l", l=PL)
                        dBu3 = dBu[:].rearrange("p (n l) -> p n l", l=PL)
                        if m == 0 and nb < 2:
                            # gap columns stay 0 across slot reuse (2 slots/tag)
                            nc.vector.memset(dA3[:, :, L:PL], 0.0)
                            nc.vector.memset(dBu3[:, :, L:PL], 0.0)
                        nc.vector.tensor_mul(
                            dBu3[:, :, 0:L],
                            u16[:, None, :].broadcast_to([128, NPB, L]),
                            bca3[:, nb * NPB:(nb + 1) * NPB, :])
                        h4 = pcp.tile([128, NPB * PL], f16, tag="h4")
                        nc.vector.tensor_tensor_scan(h4[:], dA[:], dBu[:], 0.0,
                                                     Alu.mult, Alu.add)
                        h43 = h4[:].rearrange("p (n l) -> p n l", l=PL)
                        prod = pcp.tile([128, NPB * PL], f16, tag="dBu")
                        prod3 = prod[:].rearrange("p (n l) -> p n l", l=PL)
                        nc.vector.tensor_mul(prod3[:, :, 0:L], h43[:, :, 0:L],
                                             bcc3[:, nb * NPB:(nb + 1) * NPB, :])
                        nc.vector.tensor_add(yparts[:, nb * L:(nb + 1) * L],
                                             prod[:, 0:L], prod[:, PL:PL + L])
                    t4 = pc1.tile([128, 4 * L], f16, tag="t4")
                    nc.vector.tensor_add(t4[:], yparts[:, 0:4 * L], yparts[:, 4 * L:8 * L])
                    t2 = pc1.tile([128, 2 * L], f16, tag="t2")
                    nc.vector.tensor_add(t2[:], t4[:, 0:2 * L], t4[:, 2 * L:4 * L])
                    y16 = pc1.tile([128, L], f16, tag="y16")
                    nc.vector.tensor_add(y16[:], t2[:, 0:L], t2[:, L:2 * L])
                    ys16 = pc1.tile([128, L], f16, tag="ys16")
                    nc.vector.scalar_tensor_tensor(ys16[:], xc16[:, m * L:(m + 1) * L],
                                                   dp[:, m:m + 1], y16[:], Alu.mult, Alu.add)
                    nc.vector.tensor_mul(g16[:, m * L:(m + 1) * L], ys16[:],
                                         sz16[:, m * L:(m + 1) * L])

            # ---------- Phase D: out_proj + flip-combine + AllReduce ----------
            with tc.tile_pool(name="pd", bufs=2) as pdp, \
                 tc.tile_pool(name="psD", bufs=2, space="PSUM") as psD:
                for dm in range(M_TILES):
                    wo = pdp.tile([128, M_TILES * 128], f16, tag="wo")
                    nc.sync.dma_start(wo[:], woT_d[:, dm * D:(dm + 1) * D])
                    ps_o = psD.tile([128, L], f32, tag="o")
                    for m in range(M_TILES):
                        for h in range(2):
                            nc.tensor.matmul(ps_o[:, h * 512:(h + 1) * 512],
                                             wo[:, m * 128:(m + 1) * 128],
                                             g16[:, m * L + h * 512: m * L + (h + 1) * 512],
                                             start=(m == 0), stop=(m == M_TILES - 1))
                    o32 = pdp.tile([128, L], f32, tag="o32")
                    nc.vector.tensor_copy(o32[:], ps_o[:])
                    t1 = pdp.tile([128, L], f16, tag="t1")
                    nc.vector.tensor_scalar_mul(t1[:], o32[:], mf[:, 0:1])
                    ocs = pdp.tile([128, L], f16, tag="ocs")
                    nc.vector.scalar_tensor_tensor(ocs[:], o32[:, ::-1], mb[:, 0:1],
                                                   t1[:], Alu.mult, Alu.add)
                    nc.sync.dma_start(oc_in[dm * 128:(dm + 1) * 128, :], ocs[:])
                    if dm % 2 == 1:
                        r0, r1 = (dm - 1) * 128, (dm + 1) * 128
                        nc.gpsimd.collective_compute(
                            "AllReduce", Alu.add,
                            replica_groups=[[0, 1, 2, 3], [4, 5, 6, 7]],
                            ins=[oc_in[r0:r1, :]], outs=[oc_out[r0:r1, :]])
                        nc.sync.dma_start(out_d[r0:r1, :], oc_out[r0:r1, :])

    nc.compile()
    return nc


def _host_prep(inputs):
    """Build the 8 per-core input maps from the full problem inputs."""
    x = np.asarray(inputs["x"], np.float32)
    merge_w = np.asarray(inputs["merge_w"], np.float32)
    in_maps = []
    for b in range(B):
        for di, pre in enumerate(("fwd", "bwd")):
            p = {k: np.asarray(inputs[f"{pre}_{k}"], np.float32)
                 for k in ("in_proj", "conv_w", "conv_b", "x_proj", "dt_w",
                           "dt_b", "A_log", "D", "out_proj")}
            xb = x[b]
            if di == 1:
                xb = xb[::-1]
            xTp = np.concatenate([np.zeros((D, 3), np.float32), xb.T], axis=1)
            A = -np.exp(p["A_log"])                       # (E, N)
            W = merge_w[:, di * D:(di + 1) * D] @ p["out_proj"]   # (D, E)
            def pack_lhsT(wT):
                # (D, EH) -> [p, m*1024 + kt*128 + e']
                return np.ascontiguousarray(
                    wT.reshape(M_TILES, 128, M_TILES, 128).transpose(1, 2, 0, 3)
                    .reshape(128, M_TILES * EH))

            for half in range(2):
                sl = slice(half * EH, (half + 1) * EH)
                wxiT = pack_lhsT(p["in_proj"][:E][sl].T)
                wzT = pack_lhsT(p["in_proj"][E:][sl].T)
                convw = p["conv_w"][sl].reshape(M_TILES, 128, K).transpose(1, 0, 2).reshape(128, M_TILES * K)
                convb = p["conv_b"][sl].reshape(M_TILES, 128).T
                xpT = p["x_proj"][:, sl].T                # (EH, 96)
                dtwT = p["dt_w"][sl].T                    # (DTR, EH)
                dtb = p["dt_b"][sl].reshape(M_TILES, 128).T
                arate = A[sl].reshape(M_TILES, 128, N).transpose(1, 0, 2).reshape(128, M_TILES * N)
                dpv = p["D"][sl].reshape(M_TILES, 128).T
                woT = pack_lhsT(W[:, sl].T)               # (EH, D) pre-tiled
                fwd = (di == 0)
                in_maps.append({
                    "xT": xTp.astype(np.float16),
                    "wxiT": wxiT.astype(np.float16),
                    "wzT": wzT.astype(np.float16),
                    "convw": np.ascontiguousarray(convw, np.float32),
                    "convb": np.ascontiguousarray(convb, np.float32),
                    "xpT": xpT.astype(np.float16),
                    "dtwT": np.ascontiguousarray(dtwT, np.float32),
                    "dtb": np.ascontiguousarray(dtb, np.float32),
                    "arate": np.ascontiguousarray(arate, np.float32),
                    "dp": np.ascontiguousarray(dpv, np.float32),
                    "woT": woT.astype(np.float16),
                    "mf": np.full((128, 1), 1.0 if fwd else 0.0, np.float32),
                    "mb": np.full((128, 1), 0.0 if fwd else 1.0, np.float32),
                })
    return in_maps


def _ensure_neuron_platform():
    """If a caller pinned jax to cpu, re-point it at the neuron/axon PJRT
    platform so run_bass_kernel_spmd sees the 8 NeuronCores."""
    import jax
    try:
        if len(jax.devices()) >= 8 and jax.devices()[0].platform != "cpu":
            return
    except Exception:
        pass
    for plat in ("axon", "neuron"):
        try:
            jax.config.update("jax_platforms", plat)
            if len(jax.devices()) >= 8:
                return
        except Exception:
            continue


def kernel(**inputs):
    _ensure_neuron_platform()
    from concourse.bass_utils import run_bass_kernel_spmd
    if "nc" not in _nc_cache:
        _nc_cache["nc"] = _build_nc()
    nc = _nc_cache["nc"]
    in_maps = _host_prep(inputs)
    res = run_bass_kernel_spmd(nc, in_maps, core_ids=list(range(8)))
    _nc_cache["last_results"] = res
    out = np.stack([res.results[0]["out_p"].T, res.results[4]["out_p"].T])
    return out.astype(np.float32)


# revision 27
# speedup vs baseline: 1.1829x; 1.0395x over previous
"""BiMamba (fwd+bwd Mamba + merge) Trainium2 Bass kernel.

Sharding (8 cores): core = batch*4 + dir*2 + e_half.
Each core computes one (batch, direction) pair over 1024 of the 2048 d_inner
channels, in e-partition layout [e_p=128 x 8 tiles, t_free=1024].
bwd cores operate entirely in flipped time (host pre-flips x); the final
out_proj partial is un-flipped via a data-driven mask combine, then a 4-core
AllReduce produces the full (d, t) output on every core of the batch group.

Self-contained: hardcodes B=2, L=1024, D=1024, E=2048 (1024/core), N=16,
dt_rank=64, d_conv=4.
"""
import numpy as np

B, L, D = 2, 1024, 1024
E = 2048
EH = 1024            # channels per core (half of E)
N = 16
DTR = 64
K = 4                # d_conv
M_TILES = 8          # e-tiles per core
NB = 8               # n-plane batches
NPB = 2              # planes per batch
PL = L + 2           # plane stride with 2-col zero gap for the batched scan

_nc_cache = {}


def _build_nc():
    import concourse.bacc as bacc
    import concourse.mybir as mybir
    from concourse import tile

    f32, f16 = mybir.dt.float32, mybir.dt.float16
    Alu = mybir.AluOpType
    Act = mybir.ActivationFunctionType

    nc = bacc.Bacc("TRN2", target_bir_lowering=False, debug=False, num_devices=8)

    # ---- DRAM I/O ----
    xT_d = nc.dram_tensor("xT", [D, 3 + L], f16, kind="ExternalInput")
    # pre-tiled: [p, m*1024 + kt*128 + e']  (one DMA per m-slab)
    wxiT_d = nc.dram_tensor("wxiT", [128, M_TILES * EH], f16, kind="ExternalInput")
    wzT_d = nc.dram_tensor("wzT", [128, M_TILES * EH], f16, kind="ExternalInput")
    convw_d = nc.dram_tensor("convw", [128, M_TILES * K], f32, kind="ExternalInput")
    convb_d = nc.dram_tensor("convb", [128, M_TILES], f32, kind="ExternalInput")
    xpT_d = nc.dram_tensor("xpT", [EH, 96], f16, kind="ExternalInput")
    dtwT_d = nc.dram_tensor("dtwT", [DTR, EH], f32, kind="ExternalInput")
    dtb_d = nc.dram_tensor("dtb", [128, M_TILES], f32, kind="ExternalInput")
    arate_d = nc.dram_tensor("arate", [128, M_TILES * N], f32, kind="ExternalInput")
    dp_d = nc.dram_tensor("dp", [128, M_TILES], f32, kind="ExternalInput")
    # pre-tiled: [p, dm*1024 + m*128 + d']
    woT_d = nc.dram_tensor("woT", [128, M_TILES * D], f16, kind="ExternalInput")
    mf_d = nc.dram_tensor("mf", [128, 1], f32, kind="ExternalInput")
    mb_d = nc.dram_tensor("mb", [128, 1], f32, kind="ExternalInput")

    dbl_in = nc.dram_tensor("dbl_in", [64, L], f32, kind="Internal")
    dbl_out = nc.dram_tensor("dbl_out", [64, L], f32, kind="Internal")
    bc16_in = nc.dram_tensor("bc16_in", [32, L], f16, kind="Internal")
    bc16_d = nc.dram_tensor("bc16", [32, L], f16, kind="Internal")
    oc_in = nc.dram_tensor("oc_in", [D, L], f16, kind="Internal")
    oc_out = nc.dram_tensor("oc_out", [256, L], f16, kind="Internal")
    out_d = nc.dram_tensor("out_p", [256, L], f16, kind="ExternalOutput")

    with tile.TileContext(nc) as tc:
        with tc.tile_pool(name="const", bufs=1) as cpool, \
             tc.tile_pool(name="res", bufs=1) as rpool:
            convw = cpool.tile([128, M_TILES * K], f32)
            convb = cpool.tile([128, M_TILES], f32)
            dtb = cpool.tile([128, M_TILES], f32)
            arate = cpool.tile([128, M_TILES * N], f32)
            dp = cpool.tile([128, M_TILES], f32)
            mf = cpool.tile([128, 1], f32)
            mb = cpool.tile([128, 1], f32)
            for t_, d_ in ((convw, convw_d), (convb, convb_d), (dtb, dtb_d),
                           (arate, arate_d), (dp, dp_d), (mf, mf_d), (mb, mb_d)):
                nc.sync.dma_start(t_[:], d_[:])

            xc16 = rpool.tile([128, M_TILES * L], f16)
            sz16 = rpool.tile([128, M_TILES * L], f16)
            g16 = rpool.tile([128, M_TILES * L], f16)
            bca = rpool.tile([128, N * L], f16)
            bcc = rpool.tile([128, N * L], f16)
            dblr = rpool.tile([64, L], f32)

            # ---------- Phase A: in_proj matmuls + conv + silu ----------
            with tc.tile_pool(name="pa", bufs=1) as pap, \
                 tc.tile_pool(name="paw", bufs=4) as pwp, \
                 tc.tile_pool(name="pax", bufs=2) as pxp, \
                 tc.tile_pool(name="psA", bufs=2, space="PSUM") as psA:
                xT = pap.tile([128, M_TILES * (3 + L)], f16)
                for kt in range(M_TILES):
                    nc.sync.dma_start(xT[:, kt * (3 + L):(kt + 1) * (3 + L)],
                                      xT_d[kt * 128:(kt + 1) * 128, :])
                for m in range(M_TILES):
                    wxi = pwp.tile([128, M_TILES * 128], f16, tag="wxi")
                    wz = pwp.tile([128, M_TILES * 128], f16, tag="wz")
                    nc.sync.dma_start(wxi[:], wxiT_d[:, m * EH:(m + 1) * EH])
                    nc.sync.dma_start(wz[:], wzT_d[:, m * EH:(m + 1) * EH])
                    ps_xi = psA.tile([128, L], f32, tag="xi")
                    ps_z = psA.tile([128, L], f32, tag="z")
                    for kt in range(M_TILES):
                        xk = xT[:, kt * (3 + L):(kt + 1) * (3 + L)]
                        for h in range(2):
                            nc.tensor.matmul(ps_xi[:, h * 512:(h + 1) * 512],
                                             wxi[:, kt * 128:(kt + 1) * 128],
                                             xk[:, 3 + h * 512: 3 + (h + 1) * 512],
                                             start=(kt == 0), stop=(kt == M_TILES - 1))
                            nc.tensor.matmul(ps_z[:, h * 512:(h + 1) * 512],
                                             wz[:, kt * 128:(kt + 1) * 128],
                                             xk[:, 3 + h * 512: 3 + (h + 1) * 512],
                                             start=(kt == 0), stop=(kt == M_TILES - 1))
                    # conv: xi32 padded copy, then 4-tap chain on DVE
                    xi32 = pxp.tile([128, 3 + L], f32, tag="xi32")
                    nc.vector.memset(xi32[:, 0:3], 0.0)
                    nc.scalar.copy(xi32[:, 3:3 + L], ps_xi[:])
                    cacc = pxp.tile([128, L], f32, tag="cacc")
                    nc.vector.tensor_scalar_mul(cacc[:], xi32[:, 0:L], convw[:, m * K:m * K + 1])
                    for k in range(1, K):
                        nc.vector.scalar_tensor_tensor(
                            cacc[:], xi32[:, k:k + L], convw[:, m * K + k:m * K + k + 1],
                            cacc[:], Alu.mult, Alu.add)
                    nc.scalar.activation(xc16[:, m * L:(m + 1) * L], cacc[:],
                                         Act.Silu, bias=convb[:, m:m + 1])
                    nc.scalar.activation(sz16[:, m * L:(m + 1) * L], ps_z[:], Act.Silu)

            # ---------- Phase B: x_proj partial + AllReduce + broadcasts ----------
            with tc.tile_pool(name="pb", bufs=1) as pbp, \
                 tc.tile_pool(name="pbw", bufs=2) as pbw, \
                 tc.tile_pool(name="psB", bufs=1, space="PSUM") as psB:
                ps_dbl = psB.tile([96, L], f32)
                for m in range(M_TILES):
                    xp = pbw.tile([128, 96], f16, tag="xp")
                    nc.sync.dma_start(xp[:], xpT_d[m * 128:(m + 1) * 128, :])
                    for h in range(2):
                        nc.tensor.matmul(ps_dbl[:, h * 512:(h + 1) * 512], xp[:],
                                         xc16[:, m * L + h * 512: m * L + (h + 1) * 512],
                                         start=(m == 0), stop=(m == M_TILES - 1))
                # split: dt rows AllReduce in f32; B/C rows in f16 (feeds planes)
                dbl_sb = pbp.tile([64, L], f32)
                nc.vector.tensor_copy(dbl_sb[:], ps_dbl[0:64, :])
                nc.sync.dma_start(dbl_in[:], dbl_sb[:])
                cvt16 = pbp.tile([128, L], f16)
                nc.vector.tensor_copy(cvt16[64:96, :], ps_dbl[64:96, :])
                nc.scalar.dma_start(bc16_in[:], cvt16[64:96, :])
                nc.gpsimd.collective_compute(
                    "AllReduce", Alu.add,
                    replica_groups=[[0, 1], [2, 3], [4, 5], [6, 7]],
                    ins=[bc16_in[:]], outs=[bc16_d[:]])
                nc.gpsimd.collective_compute(
                    "AllReduce", Alu.add,
                    replica_groups=[[0, 1], [2, 3], [4, 5], [6, 7]],
                    ins=[dbl_in[:]], outs=[dbl_out[:]])
                nc.sync.dma_start(dblr[0:64, :], dbl_out[:])
                engs = [nc.sync, nc.scalar, nc.gpsimd]
                for n in range(N):
                    engs[n % 3].dma_start(bca[:, n * L:(n + 1) * L],
                                          bc16_d[n:n + 1, :].broadcast_to([128, L]))
                    engs[(n + 1) % 3].dma_start(bcc[:, n * L:(n + 1) * L],
                                                bc16_d[N + n:N + n + 1, :].broadcast_to([128, L]))

            # ---------- Phase C: delta, dA planes, scan, y ----------
            with tc.tile_pool(name="pc", bufs=2) as pcp, \
                 tc.tile_pool(name="pc1", bufs=1) as pc1, \
                 tc.tile_pool(name="psC", bufs=2, space="PSUM") as psC:
                bca3 = bca[:].rearrange("p (n l) -> p n l", l=L)
                bcc3 = bcc[:].rearrange("p (n l) -> p n l", l=L)
                for m in range(M_TILES):
                    dtw = pcp.tile([DTR, 128], f32, tag="dtw")
                    nc.sync.dma_start(dtw[:], dtwT_d[:, m * 128:(m + 1) * 128])
                    ps_dt = psC.tile([128, L], f32, tag="dt")
                    for h in range(2):
                        nc.tensor.matmul(ps_dt[:, h * 512:(h + 1) * 512], dtw[:],
                                         dblr[0:DTR, h * 512:(h + 1) * 512],
                                         start=True, stop=True)
                    # softplus(raw) = ln(1 + exp(raw)); Softplus has no act table here
                    delta32 = pcp.tile([128, L], f32, tag="d32")
                    delta16 = pcp.tile([128, L], f16, tag="d16")
                    ee = pcp.tile([128, L], f32, tag="ee")
                    nc.scalar.activation(ee[:], ps_dt[:], Act.Exp, bias=dtb[:, m:m + 1])
                    nc.scalar.activation(delta32[:], ee[:], Act.Ln, bias=1.0)
                    nc.vector.tensor_copy(delta16[:], delta32[:])
                    u16 = pcp.tile([128, L], f16, tag="u16")
                    nc.vector.tensor_mul(u16[:], delta16[:], xc16[:, m * L:(m + 1) * L])
                    yparts = pc1.tile([128, NB * L], f16, tag="yp")
                    for nb in range(NB):
                        dA = pcp.tile([128, NPB * PL], f32, tag="dA")
                        dBu = pcp.tile([128, NPB * PL], f16, tag="dBu")
                        for j in range(NPB):
                            n = nb * NPB + j
                            nc.scalar.activation(dA[:, j * PL:j * PL + L], delta32[:],
                                                 Act.Exp, scale=arate[:, m * N + n:m * N + n + 1])
                        dA3 = dA[:].rearrange("p (n l) -> p n l", l=PL)
                        dBu3 = dBu[:].rearrange("p (n l) -> p n l", l=PL)
                        if m == 0 and nb < 2:
                            # gap columns stay 0 across slot reuse (2 slots/tag)
                            nc.vector.memset(dA3[:, :, L:PL], 0.0)
                            nc.vector.memset(dBu3[:, :, L:PL], 0.0)
                        nc.vector.tensor_mul(
                            dBu3[:, :, 0:L],
                            u16[:, None, :].broadcast_to([128, NPB, L]),
                            bca3[:, nb * NPB:(nb + 1) * NPB, :])
                        h4 = pcp.tile([128, NPB * PL], f16, tag="h4")
                        nc.vector.tensor_tensor_scan(h4[:], dA[:], dBu[:], 0.0,
                                                     Alu.mult, Alu.add)
                        h43 = h4[:].rearrange("p (n l) -> p n l", l=PL)
                        prod = pcp.tile([128, NPB * PL], f16, tag="dBu")
                        prod3 = prod[:].rearrange("p (n l) -> p n l", l=PL)
                        nc.vector.tensor_mul(prod3[:, :, 0:L], h43[:, :, 0:L],
                                             bcc3[:, nb * NPB:(nb + 1) * NPB, :])
                        nc.vector.tensor_add(yparts[:, nb * L:(nb + 1) * L],
                                             prod[:, 0:L], prod[:, PL:PL + L])
                    t4 = pc1.tile([128, 4 * L], f16, tag="t4")
                    nc.vector.tensor_add(t4[:], yparts[:, 0:4 * L], yparts[:, 4 * L:8 * L])
                    t2 = pc1.tile([128, 2 * L], f16, tag="t2")
                    nc.vector.tensor_add(t2[:], t4[:, 0:2 * L], t4[:, 2 * L:4 * L])
                    y16 = pc1.tile([128, L], f16, tag="y16")
                    nc.vector.tensor_add(y16[:], t2[:, 0:L], t2[:, L:2 * L])
                    ys16 = pc1.tile([128, L], f16, tag="ys16")
                    nc.vector.scalar_tensor_tensor(ys16[:], xc16[:, m * L:(m + 1) * L],
                                                   dp[:, m:m + 1], y16[:], Alu.mult, Alu.add)
                    nc.vector.tensor_mul(g16[:, m * L:(m + 1) * L], ys16[:],
                                         sz16[:, m * L:(m + 1) * L])

            # ---------- Phase D: out_proj + flip-combine + AllReduce ----------
            with tc.tile_pool(name="pd", bufs=2) as pdp, \
                 tc.tile_pool(name="psD", bufs=2, space="PSUM") as psD:
                for dm in range(M_TILES):
                    wo = pdp.tile([128, M_TILES * 128], f16, tag="wo")
                    nc.sync.dma_start(wo[:], woT_d[:, dm * D:(dm + 1) * D])
                    ps_o = psD.tile([128, L], f32, tag="o")
                    for m in range(M_TILES):
                        for h in range(2):
                            nc.tensor.matmul(ps_o[:, h * 512:(h + 1) * 512],
                                             wo[:, m * 128:(m + 1) * 128],
                                             g16[:, m * L + h * 512: m * L + (h + 1) * 512],
                                             start=(m == 0), stop=(m == M_TILES - 1))
                    o32 = pdp.tile([128, L], f32, tag="o32")
                    nc.vector.tensor_copy(o32[:], ps_o[:])
                    t1 = pdp.tile([128, L], f16, tag="t1")
                    nc.vector.tensor_scalar_mul(t1[:], o32[:], mf[:, 0:1])
                    ocs = pdp.tile([128, L], f16, tag="ocs")
                    nc.vector.scalar_tensor_tensor(ocs[:], o32[:, ::-1], mb[:, 0:1],
                                                   t1[:], Alu.mult, Alu.add)
                    nc.sync.dma_start(oc_in[dm * 128:(dm + 1) * 128, :], ocs[:])
                    if dm % 4 == 3:
                        # ReduceScatter: group-rank ci gets a contiguous 128-row
                        # shard of the 512-row chunk; host stitches quarters.
                        ch = dm // 4
                        nc.gpsimd.collective_compute(
                            "ReduceScatter", Alu.add,
                            replica_groups=[[0, 1, 2, 3], [4, 5, 6, 7]],
                            ins=[oc_in[ch * 512:(ch + 1) * 512, :]],
                            outs=[oc_out[ch * 128:(ch + 1) * 128, :]])
                        nc.sync.dma_start(out_d[ch * 128:(ch + 1) * 128, :],
                                          oc_out[ch * 128:(ch + 1) * 128, :])

    nc.compile()
    return nc


def _host_prep(inputs):
    """Build the 8 per-core input maps from the full problem inputs."""
    x = np.asarray(inputs["x"], np.float32)
    merge_w = np.asarray(inputs["merge_w"], np.float32)
    in_maps = []
    for b in range(B):
        for di, pre in enumerate(("fwd", "bwd")):
            p = {k: np.asarray(inputs[f"{pre}_{k}"], np.float32)
                 for k in ("in_proj", "conv_w", "conv_b", "x_proj", "dt_w",
                           "dt_b", "A_log", "D", "out_proj")}
            xb = x[b]
            if di == 1:
                xb = xb[::-1]
            xTp = np.concatenate([np.zeros((D, 3), np.float32), xb.T], axis=1)
            A = -np.exp(p["A_log"])                       # (E, N)
            W = merge_w[:, di * D:(di + 1) * D] @ p["out_proj"]   # (D, E)
            def pack_lhsT(wT):
                # (D, EH) -> [p, m*1024 + kt*128 + e']
                return np.ascontiguousarray(
                    wT.reshape(M_TILES, 128, M_TILES, 128).transpose(1, 2, 0, 3)
                    .reshape(128, M_TILES * EH))

            for half in range(2):
                sl = slice(half * EH, (half + 1) * EH)
                wxiT = pack_lhsT(p["in_proj"][:E][sl].T)
                wzT = pack_lhsT(p["in_proj"][E:][sl].T)
                convw = p["conv_w"][sl].reshape(M_TILES, 128, K).transpose(1, 0, 2).reshape(128, M_TILES * K)
                convb = p["conv_b"][sl].reshape(M_TILES, 128).T
                xpT = p["x_proj"][:, sl].T                # (EH, 96)
                dtwT = p["dt_w"][sl].T                    # (DTR, EH)
                dtb = p["dt_b"][sl].reshape(M_TILES, 128).T
                arate = A[sl].reshape(M_TILES, 128, N).transpose(1, 0, 2).reshape(128, M_TILES * N)
                dpv = p["D"][sl].reshape(M_TILES, 128).T
                woT = pack_lhsT(W[:, sl].T)               # (EH, D) pre-tiled
                fwd = (di == 0)
                in_maps.append({
                    "xT": xTp.astype(np.float16),
                    "wxiT": wxiT.astype(np.float16),
                    "wzT": wzT.astype(np.float16),
                    "convw": np.ascontiguousarray(convw, np.float32),
                    "convb": np.ascontiguousarray(convb, np.float32),
                    "xpT": xpT.astype(np.float16),
                    "dtwT": np.ascontiguousarray(dtwT, np.float32),
                    "dtb": np.ascontiguousarray(dtb, np.float32),
                    "arate": np.ascontiguousarray(arate, np.float32),
                    "dp": np.ascontiguousarray(dpv, np.float32),
                    "woT": woT.astype(np.float16),
                    "mf": np.full((128, 1), 1.0 if fwd else 0.0, np.float32),
                    "mb": np.full((128, 1), 0.0 if fwd else 1.0, np.float32),
                })
    return in_maps


def _ensure_neuron_platform():
    """If a caller pinned jax to cpu, re-point it at the neuron/axon PJRT
    platform so run_bass_kernel_spmd sees the 8 NeuronCores."""
    import jax
    try:
        if len(jax.devices()) >= 8 and jax.devices()[0].platform != "cpu":
            return
    except Exception:
        pass
    for plat in ("axon", "neuron"):
        try:
            jax.config.update("jax_platforms", plat)
            if len(jax.devices()) >= 8:
                return
        except Exception:
            continue


def kernel(**inputs):
    _ensure_neuron_platform()
    from concourse.bass_utils import run_bass_kernel_spmd
    if "nc" not in _nc_cache:
        _nc_cache["nc"] = _build_nc()
    nc = _nc_cache["nc"]
    in_maps = _host_prep(inputs)
    res = run_bass_kernel_spmd(nc, in_maps, core_ids=list(range(8)))
    _nc_cache["last_results"] = res
    # Stitch ReduceScatter shards: core 4b+ci holds d-rows [ci*128:(ci+1)*128]
    # (chunk 0, out_p rows 0:128) and [512+ci*128 : 512+(ci+1)*128] (chunk 1).
    out = np.zeros((B, L, D), np.float32)
    for b in range(B):
        od = np.zeros((D, L), np.float32)
        for ci in range(4):
            shard = res.results[4 * b + ci]["out_p"].astype(np.float32)
            od[ci * 128:(ci + 1) * 128] = shard[0:128]
            od[512 + ci * 128:512 + (ci + 1) * 128] = shard[128:256]
        out[b] = od.T
    return out


# revision 30
# speedup vs baseline: 1.1918x; 1.0076x over previous
"""BiMamba (fwd+bwd Mamba + merge) Trainium2 Bass kernel.

Sharding (8 cores): core = batch*4 + dir*2 + e_half.
Each core computes one (batch, direction) pair over 1024 of the 2048 d_inner
channels, in e-partition layout [e_p=128 x 8 tiles, t_free=1024].
bwd cores operate entirely in flipped time (host pre-flips x); the final
out_proj partial is un-flipped via a data-driven mask combine, then a 4-core
AllReduce produces the full (d, t) output on every core of the batch group.

Self-contained: hardcodes B=2, L=1024, D=1024, E=2048 (1024/core), N=16,
dt_rank=64, d_conv=4.
"""
import numpy as np

B, L, D = 2, 1024, 1024
E = 2048
EH = 1024            # channels per core (half of E)
N = 16
DTR = 64
K = 4                # d_conv
M_TILES = 8          # e-tiles per core
NB = 8               # n-plane batches
NPB = 2              # planes per batch
PL = L + 2           # plane stride with 2-col zero gap for the batched scan

_nc_cache = {}


def _build_nc():
    import concourse.bacc as bacc
    import concourse.mybir as mybir
    from concourse import tile

    f32, f16 = mybir.dt.float32, mybir.dt.float16
    Alu = mybir.AluOpType
    Act = mybir.ActivationFunctionType

    nc = bacc.Bacc("TRN2", target_bir_lowering=False, debug=False, num_devices=8)

    # ---- DRAM I/O ----
    xT_d = nc.dram_tensor("xT", [D, 3 + L], f16, kind="ExternalInput")
    # pre-tiled: [p, m*1024 + kt*128 + e']  (one DMA per m-slab)
    wxiT_d = nc.dram_tensor("wxiT", [128, M_TILES * EH], f16, kind="ExternalInput")
    wzT_d = nc.dram_tensor("wzT", [128, M_TILES * EH], f16, kind="ExternalInput")
    convw_d = nc.dram_tensor("convw", [128, M_TILES * K], f32, kind="ExternalInput")
    convb_d = nc.dram_tensor("convb", [128, M_TILES], f32, kind="ExternalInput")
    xpT_d = nc.dram_tensor("xpT", [EH, 96], f16, kind="ExternalInput")
    dtwT_d = nc.dram_tensor("dtwT", [DTR, EH], f32, kind="ExternalInput")
    dtb_d = nc.dram_tensor("dtb", [128, M_TILES], f32, kind="ExternalInput")
    arate_d = nc.dram_tensor("arate", [128, M_TILES * N], f32, kind="ExternalInput")
    dp_d = nc.dram_tensor("dp", [128, M_TILES], f32, kind="ExternalInput")
    # pre-tiled: [p, dm*1024 + m*128 + d']
    woT_d = nc.dram_tensor("woT", [128, M_TILES * D], f16, kind="ExternalInput")
    mf_d = nc.dram_tensor("mf", [128, 1], f32, kind="ExternalInput")
    mb_d = nc.dram_tensor("mb", [128, 1], f32, kind="ExternalInput")

    dbl_in = nc.dram_tensor("dbl_in", [64, L], f32, kind="Internal")
    dbl_out = nc.dram_tensor("dbl_out", [64, L], f32, kind="Internal")
    bc16_in = nc.dram_tensor("bc16_in", [32, L], f16, kind="Internal")
    bc16_d = nc.dram_tensor("bc16", [32, L], f16, kind="Internal")
    oc_in = nc.dram_tensor("oc_in", [D, L], f16, kind="Internal")
    oc_out = nc.dram_tensor("oc_out", [256, L], f16, kind="Internal")
    out_d = nc.dram_tensor("out_p", [256, L], f16, kind="ExternalOutput")

    with tile.TileContext(nc) as tc:
        with tc.tile_pool(name="const", bufs=1) as cpool, \
             tc.tile_pool(name="res", bufs=1) as rpool:
            convw = cpool.tile([128, M_TILES * K], f32)
            convb = cpool.tile([128, M_TILES], f32)
            dtb = cpool.tile([128, M_TILES], f32)
            arate = cpool.tile([128, M_TILES * N], f32)
            dp = cpool.tile([128, M_TILES], f32)
            mf = cpool.tile([128, 1], f32)
            mb = cpool.tile([128, 1], f32)
            for t_, d_ in ((convw, convw_d), (convb, convb_d), (dtb, dtb_d),
                           (arate, arate_d), (dp, dp_d), (mf, mf_d), (mb, mb_d)):
                nc.sync.dma_start(t_[:], d_[:])

            xc16 = rpool.tile([128, M_TILES * L], f16)
            sz16 = rpool.tile([128, M_TILES * L], f16)
            g16 = rpool.tile([128, M_TILES * L], f16)
            bca = rpool.tile([128, N * L], f16)
            bcc = rpool.tile([128, N * L], f16)
            dblr = rpool.tile([64, L], f32)

            # ---------- Phase A: in_proj matmuls + conv + silu ----------
            with tc.tile_pool(name="pa", bufs=1) as pap, \
                 tc.tile_pool(name="paw", bufs=4) as pwp, \
                 tc.tile_pool(name="pax", bufs=2) as pxp, \
                 tc.tile_pool(name="psA", bufs=2, space="PSUM") as psA:
                xT = pap.tile([128, M_TILES * (3 + L)], f16)
                dma_engs = [nc.sync, nc.scalar, nc.gpsimd]
                for kt in range(M_TILES):
                    dma_engs[kt % 3].dma_start(xT[:, kt * (3 + L):(kt + 1) * (3 + L)],
                                               xT_d[kt * 128:(kt + 1) * 128, :])
                for m in range(M_TILES):
                    wxi = pwp.tile([128, M_TILES * 128], f16, tag="wxi")
                    wz = pwp.tile([128, M_TILES * 128], f16, tag="wz")
                    nc.scalar.dma_start(wxi[:], wxiT_d[:, m * EH:(m + 1) * EH])
                    nc.gpsimd.dma_start(wz[:], wzT_d[:, m * EH:(m + 1) * EH])
                    ps_xi = psA.tile([128, L], f32, tag="xi")
                    ps_z = psA.tile([128, L], f32, tag="z")
                    for kt in range(M_TILES):
                        xk = xT[:, kt * (3 + L):(kt + 1) * (3 + L)]
                        for h in range(2):
                            nc.tensor.matmul(ps_xi[:, h * 512:(h + 1) * 512],
                                             wxi[:, kt * 128:(kt + 1) * 128],
                                             xk[:, 3 + h * 512: 3 + (h + 1) * 512],
                                             start=(kt == 0), stop=(kt == M_TILES - 1))
                            nc.tensor.matmul(ps_z[:, h * 512:(h + 1) * 512],
                                             wz[:, kt * 128:(kt + 1) * 128],
                                             xk[:, 3 + h * 512: 3 + (h + 1) * 512],
                                             start=(kt == 0), stop=(kt == M_TILES - 1))
                    # conv: xi32 padded copy, then 4-tap chain on DVE
                    xi32 = pxp.tile([128, 3 + L], f32, tag="xi32")
                    nc.vector.memset(xi32[:, 0:3], 0.0)
                    nc.scalar.copy(xi32[:, 3:3 + L], ps_xi[:])
                    cacc = pxp.tile([128, L], f32, tag="cacc")
                    nc.vector.tensor_scalar_mul(cacc[:], xi32[:, 0:L], convw[:, m * K:m * K + 1])
                    for k in range(1, K):
                        nc.vector.scalar_tensor_tensor(
                            cacc[:], xi32[:, k:k + L], convw[:, m * K + k:m * K + k + 1],
                            cacc[:], Alu.mult, Alu.add)
                    nc.scalar.activation(xc16[:, m * L:(m + 1) * L], cacc[:],
                                         Act.Silu, bias=convb[:, m:m + 1])
                    nc.scalar.activation(sz16[:, m * L:(m + 1) * L], ps_z[:], Act.Silu)

            # ---------- Phase B: x_proj partial + AllReduce + broadcasts ----------
            with tc.tile_pool(name="pb", bufs=1) as pbp, \
                 tc.tile_pool(name="pbw", bufs=2) as pbw, \
                 tc.tile_pool(name="psB", bufs=1, space="PSUM") as psB:
                ps_dbl = psB.tile([96, L], f32)
                for m in range(M_TILES):
                    xp = pbw.tile([128, 96], f16, tag="xp")
                    nc.sync.dma_start(xp[:], xpT_d[m * 128:(m + 1) * 128, :])
                    for h in range(2):
                        nc.tensor.matmul(ps_dbl[:, h * 512:(h + 1) * 512], xp[:],
                                         xc16[:, m * L + h * 512: m * L + (h + 1) * 512],
                                         start=(m == 0), stop=(m == M_TILES - 1))
                # split: dt rows AllReduce in f32; B/C rows in f16 (feeds planes)
                dbl_sb = pbp.tile([64, L], f32)
                nc.vector.tensor_copy(dbl_sb[:], ps_dbl[0:64, :])
                nc.sync.dma_start(dbl_in[:], dbl_sb[:])
                cvt16 = pbp.tile([128, L], f16)
                nc.vector.tensor_copy(cvt16[64:96, :], ps_dbl[64:96, :])
                nc.scalar.dma_start(bc16_in[:], cvt16[64:96, :])
                nc.gpsimd.collective_compute(
                    "AllReduce", Alu.add,
                    replica_groups=[[0, 1], [2, 3], [4, 5], [6, 7]],
                    ins=[bc16_in[:]], outs=[bc16_d[:]])
                nc.gpsimd.collective_compute(
                    "AllReduce", Alu.add,
                    replica_groups=[[0, 1], [2, 3], [4, 5], [6, 7]],
                    ins=[dbl_in[:]], outs=[dbl_out[:]])
                nc.sync.dma_start(dblr[0:64, :], dbl_out[:])
                engs = [nc.sync, nc.scalar, nc.gpsimd]
                for n in range(N):
                    engs[n % 3].dma_start(bca[:, n * L:(n + 1) * L],
                                          bc16_d[n:n + 1, :].broadcast_to([128, L]))
                    engs[(n + 1) % 3].dma_start(bcc[:, n * L:(n + 1) * L],
                                                bc16_d[N + n:N + n + 1, :].broadcast_to([128, L]))

            # ---------- Phase C: delta, dA planes, scan, y ----------
            with tc.tile_pool(name="pc", bufs=2) as pcp, \
                 tc.tile_pool(name="pc1", bufs=1) as pc1, \
                 tc.tile_pool(name="psC", bufs=2, space="PSUM") as psC:
                bca3 = bca[:].rearrange("p (n l) -> p n l", l=L)
                bcc3 = bcc[:].rearrange("p (n l) -> p n l", l=L)
                for m in range(M_TILES):
                    dtw = pcp.tile([DTR, 128], f32, tag="dtw")
                    nc.sync.dma_start(dtw[:], dtwT_d[:, m * 128:(m + 1) * 128])
                    ps_dt = psC.tile([128, L], f32, tag="dt")
                    for h in range(2):
                        nc.tensor.matmul(ps_dt[:, h * 512:(h + 1) * 512], dtw[:],
                                         dblr[0:DTR, h * 512:(h + 1) * 512],
                                         start=True, stop=True)
                    # softplus(raw) = ln(1 + exp(raw)); Softplus has no act table here
                    delta32 = pcp.tile([128, L], f32, tag="d32")
                    delta16 = pcp.tile([128, L], f16, tag="d16")
                    ee = pcp.tile([128, L], f32, tag="ee")
                    nc.scalar.activation(ee[:], ps_dt[:], Act.Exp, bias=dtb[:, m:m + 1])
                    nc.scalar.activation(delta32[:], ee[:], Act.Ln, bias=1.0)
                    nc.vector.tensor_copy(delta16[:], delta32[:])
                    u16 = pcp.tile([128, L], f16, tag="u16")
                    nc.vector.tensor_mul(u16[:], delta16[:], xc16[:, m * L:(m + 1) * L])
                    yparts = pc1.tile([128, NB * L], f16, tag="yp")
                    for nb in range(NB):
                        dA = pcp.tile([128, NPB * PL], f32, tag="dA")
                        dBu = pcp.tile([128, NPB * PL], f16, tag="dBu")
                        for j in range(NPB):
                            n = nb * NPB + j
                            nc.scalar.activation(dA[:, j * PL:j * PL + L], delta32[:],
                                                 Act.Exp, scale=arate[:, m * N + n:m * N + n + 1])
                        dA3 = dA[:].rearrange("p (n l) -> p n l", l=PL)
                        dBu3 = dBu[:].rearrange("p (n l) -> p n l", l=PL)
                        if m == 0 and nb < 2:
                            # gap columns stay 0 across slot reuse (2 slots/tag)
                            nc.vector.memset(dA3[:, :, L:PL], 0.0)
                            nc.vector.memset(dBu3[:, :, L:PL], 0.0)
                        nc.vector.tensor_mul(
                            dBu3[:, :, 0:L],
                            u16[:, None, :].broadcast_to([128, NPB, L]),
                            bca3[:, nb * NPB:(nb + 1) * NPB, :])
                        h4 = pcp.tile([128, NPB * PL], f16, tag="h4")
                        nc.vector.tensor_tensor_scan(h4[:], dA[:], dBu[:], 0.0,
                                                     Alu.mult, Alu.add)
                        h43 = h4[:].rearrange("p (n l) -> p n l", l=PL)
                        prod = pcp.tile([128, NPB * PL], f16, tag="dBu")
                        prod3 = prod[:].rearrange("p (n l) -> p n l", l=PL)
                        nc.vector.tensor_mul(prod3[:, :, 0:L], h43[:, :, 0:L],
                                             bcc3[:, nb * NPB:(nb + 1) * NPB, :])
                        nc.vector.tensor_add(yparts[:, nb * L:(nb + 1) * L],
                                             prod[:, 0:L], prod[:, PL:PL + L])
                    t4 = pc1.tile([128, 4 * L], f16, tag="t4")
                    nc.vector.tensor_add(t4[:], yparts[:, 0:4 * L], yparts[:, 4 * L:8 * L])
                    t2 = pc1.tile([128, 2 * L], f16, tag="t2")
                    nc.vector.tensor_add(t2[:], t4[:, 0:2 * L], t4[:, 2 * L:4 * L])
                    y16 = pc1.tile([128, L], f16, tag="y16")
                    nc.vector.tensor_add(y16[:], t2[:, 0:L], t2[:, L:2 * L])
                    ys16 = pc1.tile([128, L], f16, tag="ys16")
                    nc.vector.scalar_tensor_tensor(ys16[:], xc16[:, m * L:(m + 1) * L],
                                                   dp[:, m:m + 1], y16[:], Alu.mult, Alu.add)
                    nc.vector.tensor_mul(g16[:, m * L:(m + 1) * L], ys16[:],
                                         sz16[:, m * L:(m + 1) * L])

            # ---------- Phase D: out_proj + flip-combine + AllReduce ----------
            with tc.tile_pool(name="pd", bufs=2) as pdp, \
                 tc.tile_pool(name="psD", bufs=2, space="PSUM") as psD:
                for dm in range(M_TILES):
                    wo = pdp.tile([128, M_TILES * 128], f16, tag="wo")
                    nc.sync.dma_start(wo[:], woT_d[:, dm * D:(dm + 1) * D])
                    ps_o = psD.tile([128, L], f32, tag="o")
                    for m in range(M_TILES):
                        for h in range(2):
                            nc.tensor.matmul(ps_o[:, h * 512:(h + 1) * 512],
                                             wo[:, m * 128:(m + 1) * 128],
                                             g16[:, m * L + h * 512: m * L + (h + 1) * 512],
                                             start=(m == 0), stop=(m == M_TILES - 1))
                    o32 = pdp.tile([128, L], f32, tag="o32")
                    nc.vector.tensor_copy(o32[:], ps_o[:])
                    t1 = pdp.tile([128, L], f16, tag="t1")
                    nc.vector.tensor_scalar_mul(t1[:], o32[:], mf[:, 0:1])
                    ocs = pdp.tile([128, L], f16, tag="ocs")
                    nc.vector.scalar_tensor_tensor(ocs[:], o32[:, ::-1], mb[:, 0:1],
                                                   t1[:], Alu.mult, Alu.add)
                    nc.sync.dma_start(oc_in[dm * 128:(dm + 1) * 128, :], ocs[:])
                    if dm % 2 == 1:
                        # ReduceScatter: group-rank ci gets a contiguous 64-row
                        # shard of each 256-row chunk; host stitches shards.
                        ch = dm // 2
                        nc.gpsimd.collective_compute(
                            "ReduceScatter", Alu.add,
                            replica_groups=[[0, 1, 2, 3], [4, 5, 6, 7]],
                            ins=[oc_in[ch * 256:(ch + 1) * 256, :]],
                            outs=[oc_out[ch * 64:(ch + 1) * 64, :]])
                        nc.sync.dma_start(out_d[ch * 64:(ch + 1) * 64, :],
                                          oc_out[ch * 64:(ch + 1) * 64, :])

    nc.compile()
    return nc


def _host_prep(inputs):
    """Build the 8 per-core input maps from the full problem inputs."""
    x = np.asarray(inputs["x"], np.float32)
    merge_w = np.asarray(inputs["merge_w"], np.float32)
    in_maps = []
    for b in range(B):
        for di, pre in enumerate(("fwd", "bwd")):
            p = {k: np.asarray(inputs[f"{pre}_{k}"], np.float32)
                 for k in ("in_proj", "conv_w", "conv_b", "x_proj", "dt_w",
                           "dt_b", "A_log", "D", "out_proj")}
            xb = x[b]
            if di == 1:
                xb = xb[::-1]
            xTp = np.concatenate([np.zeros((D, 3), np.float32), xb.T], axis=1)
            A = -np.exp(p["A_log"])                       # (E, N)
            W = merge_w[:, di * D:(di + 1) * D] @ p["out_proj"]   # (D, E)
            def pack_lhsT(wT):
                # (D, EH) -> [p, m*1024 + kt*128 + e']
                return np.ascontiguousarray(
                    wT.reshape(M_TILES, 128, M_TILES, 128).transpose(1, 2, 0, 3)
                    .reshape(128, M_TILES * EH))

            for half in range(2):
                sl = slice(half * EH, (half + 1) * EH)
                wxiT = pack_lhsT(p["in_proj"][:E][sl].T)
                wzT = pack_lhsT(p["in_proj"][E:][sl].T)
                convw = p["conv_w"][sl].reshape(M_TILES, 128, K).transpose(1, 0, 2).reshape(128, M_TILES * K)
                convb = p["conv_b"][sl].reshape(M_TILES, 128).T
                xpT = p["x_proj"][:, sl].T                # (EH, 96)
                dtwT = p["dt_w"][sl].T                    # (DTR, EH)
                dtb = p["dt_b"][sl].reshape(M_TILES, 128).T
                arate = A[sl].reshape(M_TILES, 128, N).transpose(1, 0, 2).reshape(128, M_TILES * N)
                dpv = p["D"][sl].reshape(M_TILES, 128).T
                woT = pack_lhsT(W[:, sl].T)               # (EH, D) pre-tiled
                fwd = (di == 0)
                in_maps.append({
                    "xT": xTp.astype(np.float16),
                    "wxiT": wxiT.astype(np.float16),
                    "wzT": wzT.astype(np.float16),
                    "convw": np.ascontiguousarray(convw, np.float32),
                    "convb": np.ascontiguousarray(convb, np.float32),
                    "xpT": xpT.astype(np.float16),
                    "dtwT": np.ascontiguousarray(dtwT, np.float32),
                    "dtb": np.ascontiguousarray(dtb, np.float32),
                    "arate": np.ascontiguousarray(arate, np.float32),
                    "dp": np.ascontiguousarray(dpv, np.float32),
                    "woT": woT.astype(np.float16),
                    "mf": np.full((128, 1), 1.0 if fwd else 0.0, np.float32),
                    "mb": np.full((128, 1), 0.0 if fwd else 1.0, np.float32),
                })
    return in_maps


def _ensure_neuron_platform():
    """If a caller pinned jax to cpu, re-point it at the neuron/axon PJRT
    platform so run_bass_kernel_spmd sees the 8 NeuronCores."""
    import jax
    try:
        if len(jax.devices()) >= 8 and jax.devices()[0].platform != "cpu":
            return
    except Exception:
        pass
    for plat in ("axon", "neuron"):
        try:
            jax.config.update("jax_platforms", plat)
            if len(jax.devices()) >= 8:
                return
        except Exception:
            continue


def kernel(**inputs):
    _ensure_neuron_platform()
    from concourse.bass_utils import run_bass_kernel_spmd
    if "nc" not in _nc_cache:
        _nc_cache["nc"] = _build_nc()
    nc = _nc_cache["nc"]
    in_maps = _host_prep(inputs)
    res = run_bass_kernel_spmd(nc, in_maps, core_ids=list(range(8)))
    _nc_cache["last_results"] = res
    # Stitch ReduceScatter shards: 4 chunks of 256 d-rows; within chunk ch,
    # group-rank ci holds rows [ch*256 + ci*64 : +64] at out_p[ch*64:(ch+1)*64].
    out = np.zeros((B, L, D), np.float32)
    for b in range(B):
        od = np.zeros((D, L), np.float32)
        for ci in range(4):
            shard = res.results[4 * b + ci]["out_p"].astype(np.float32)
            for ch in range(4):
                od[ch * 256 + ci * 64: ch * 256 + (ci + 1) * 64] = \
                    shard[ch * 64:(ch + 1) * 64]
        out[b] = od.T
    return out
